# revision 1
# baseline (speedup 1.0000x reference)
"""Multi-head self-attention (B=16,T=512,C=1024,H=16) on 8 NeuronCores.

Strategy: data-parallel over batch (2 batches/core), no collectives.
Schedule keeps the PE dense (HAM stays warm at 2.4GHz) and hides the
scalar-engine exp (the softmax) under the projection matmuls:

  pair p: QK-project head pair p -> scores + exp + mask for 4 (b,h) chains
  V projection + AV + out-projection woven between pairs so every engine
  always has work and nothing big sits at the tail.

Layouts avoid on-device transposes (same tricks as the ancestor kernel):
  - QK projection emits [f, tok]; scores are computed transposed
    sT[kt, qt]; softmax sums come from a ones-column appended to v.
  - softmax 1/l: per-head row sums parked in SBUF by scalar-engine copies,
    gathered per 8-head half with one SBUF->SBUF DMA, reciprocal'd with an
    elementwise [64,64] reshape (64 DVE lanes instead of 8), and broadcast
    back across each head's 64 data partitions with stride-0 DRAM-bounce
    DMAs; ao is drained unnormalized (PSUM freed fast) and normalized in
    place afterwards.
Engine roles: PE matmuls only; scalar softmax exp + l-row copies; vector
QK/V/ao drains, diag masks, reciprocal, normalize; sync + scalar DMA
queues split so x/weights and the l-chain never serialize behind each
other. PSUM: 2 banks QK/V/yproj + 3 scores (r1/r3 share a bank via a
start=False accumulate into the still-pending zero-region) + 3 AV.
"""

import math
from contextlib import ExitStack

import numpy as np

import concourse.bass as bass
import concourse.mybir as mybir
import concourse.tile as tile
from concourse import bacc
from concourse.bass_utils import run_bass_kernel_spmd

_DEBUG = False

N_CORES = 8
B, T, C = 16, 512, 1024
H = 16
DH = C // H  # 64
B_LOC = B // N_CORES  # 2
TOK = B_LOC * T  # 1024 tokens per core
P = 128
CT = C // P  # 8 contraction tiles
NR = T // P  # 4 kt blocks
DT = mybir.dt.float16
F32 = mybir.dt.float32

# compact pT column offsets per kt-block r (lengths 512,384,256,128)
POFF = [0, 512, 896, 1152]
PTW = 1280


def _build_nc():
    nc = bacc.Bacc("TRN2", target_bir_lowering=False, debug=False,
                   num_devices=N_CORES)

    xT = nc.dram_tensor("xT", [C, TOK], DT, kind="ExternalInput").ap()
    wqk = nc.dram_tensor("wqk", [16, P, CT, P], DT, kind="ExternalInput").ap()
    wv = nc.dram_tensor("wv", [P, 2, CT, 512], DT, kind="ExternalInput").ap()
    wo = nc.dram_tensor("wo", [P, 2, CT, 512], DT, kind="ExternalInput").ap()
    maskd = nc.dram_tensor("maskd", [NR, P, P], DT,
                           kind="ExternalInput").ap()
    kpmb = nc.dram_tensor("kpmb", [B_LOC, T], F32, kind="ExternalInput").ap()
    bias = nc.dram_tensor("bias", [C], F32, kind="ExternalInput").ap()
    linv = nc.dram_tensor("linv", [B_LOC, H, T], DT).ap()
    out = nc.dram_tensor("out", [TOK, C], DT, kind="ExternalOutput").ap()
    dbg = None
    if _DEBUG:
        dbg = {
            "dbg_qk": nc.dram_tensor("dbg_qk", [P, 16, TOK], DT,
                                     kind="ExternalOutput").ap(),
            "dbg_v": nc.dram_tensor("dbg_v", [P, TOK // P, H, DH + 1], DT,
                                    kind="ExternalOutput").ap(),
            "dbg_ao": nc.dram_tensor("dbg_ao", [B_LOC, P, CT, T], DT,
                                     kind="ExternalOutput").ap(),
        }

    with tile.TileContext(nc) as tc:
        _emit(nc, tc, xT, wqk, wv, wo, maskd, kpmb, bias, linv, out, dbg)

    nc.compile()
    return nc


def _emit(nc, tc, xT, wqk, wv, wo, maskd, kpmb, bias, linv, out,
          dbg=None):
    ctx = ExitStack()
    with ctx:
        singles = ctx.enter_context(tc.tile_pool(name="singles", bufs=1))
        ps_qk = ctx.enter_context(tc.tile_pool(name="ps_qk", bufs=2,
                                               space="PSUM"))
        ps_s = ctx.enter_context(tc.tile_pool(name="ps_s", bufs=1,
                                              space="PSUM"))
        ps_o = ctx.enter_context(tc.tile_pool(name="ps_o", bufs=3,
                                              space="PSUM"))
        wq_pool = ctx.enter_context(tc.tile_pool(name="wq", bufs=4))
        pt_pool = ctx.enter_context(tc.tile_pool(name="pt", bufs=11))
        lnx_pool = ctx.enter_context(tc.tile_pool(name="lnx", bufs=3))
        li_pool = ctx.enter_context(tc.tile_pool(name="li", bufs=2))
        lfsb_pool = ctx.enter_context(tc.tile_pool(name="lfsb", bufs=2))
        aost_pool = ctx.enter_context(tc.tile_pool(name="aost", bufs=2))
        y_pool = ctx.enter_context(tc.tile_pool(name="y", bufs=3))

        # --- persistent SBUF tensors ---
        qk_sb = singles.tile([P, 16, TOK], DT)             # 32 KB/part
        v_sb = singles.tile([P, TOK // P, H, DH + 1], DT)  # 16.6 KB/part
        ao_b = [singles.tile([P, CT, T], DT, name=f"ao_b{b}")
                for b in range(B_LOC)]                     # 2x 8 KB/part
        wv_sb = [singles.tile([P, CT, 512], DT, name=f"wv_{n}")
                 for n in range(2)]                        # 16 KB/part
        wo_sb = [singles.tile([P, CT, 512], DT, name=f"wo_{n}")
                 for n in range(2)]                        # 16 KB/part

        lgL = [singles.tile([DH, DH], DT, name=f"lgL{b}")
               for b in range(B_LOC)]
        bias_sb = singles.tile([P, C], F32)                # 4 KB/part
        maskd_sb = singles.tile([P, NR, P], DT)            # 1 KB/part
        kpmb_sb = singles.tile([P, B_LOC * NR], F32)
        xk = [singles.tile([P, TOK], DT, name=f"x_{k}") for k in range(CT)]

        # --- prologue DMAs: x split over both queues, first wq first ---
        nc.sync.dma_start(out=xk[0][:], in_=xT[0:P, :])
        wq0 = wq_pool.tile([P, CT, P], DT, tag="wq", name="wq_0")
        nc.sync.dma_start(out=wq0[:], in_=wqk[0])
        for k in range(1, CT):
            # three parallel DMA streams at startup: sync, scalar HWDGE and
            # the gpsimd software DGE each carry a share of the x tiles
            eng = (nc.gpsimd if k >= 6 else
                   nc.sync if k % 2 == 0 else nc.scalar)
            eng.dma_start(out=xk[k][:], in_=xT[k * P:(k + 1) * P, :])
        nc.scalar.dma_start(out=maskd_sb[:],
                            in_=maskd.rearrange("r p q -> p r q"))
        nc.scalar.dma_start(out=kpmb_sb[:],
                            in_=kpmb.rearrange("b (r p) -> p (b r)", p=P))
        bias_bcast = bass.AP(tensor=bias.tensor, offset=bias.offset,
                             ap=[[0, P], *bias.ap])
        nc.gpsimd.dma_start(out=bias_sb[:], in_=bias_bcast)
        nc.vector.memset(v_sb[:, :, :, DH:DH + 1], 1.0)


        def emit_qk(j, wq_tile=None):
            if wq_tile is None:
                wq_tile = wq_pool.tile([P, CT, P], DT, tag="wq",
                                       name=f"wq_{j}")
                nc.sync.dma_start(out=wq_tile[:], in_=wqk[j])
            for tt in range(2):
                ps = ps_qk.tile([P, 512], F32, tag="ps", name=f"ps_qk{j}_{tt}")
                for k in range(CT):
                    nc.tensor.matmul(ps[:], wq_tile[:, k, :],
                                     xk[k][:, tt * 512:(tt + 1) * 512],
                                     start=(k == 0), stop=(k == CT - 1))
                nc.vector.tensor_copy(
                    out=qk_sb[:, j, tt * 512:(tt + 1) * 512], in_=ps[:])

        pt_tiles = {}

        def emit_scores(b, h):
            p = h // 2
            dlo = DH * (h % 2)
            jq, jk = p, 8 + p
            pT = pt_pool.tile([P, PTW], DT, tag="pT", name=f"pT_{b}_{h}")
            pt_tiles[(b, h)] = pT
            sA = ps_s.tile([P, 512], F32, tag="sA", name=f"sA_{b}_{h}")
            sB = ps_s.tile([P, 512], F32, tag="sB", name=f"sB_{b}_{h}")
            sC = ps_s.tile([P, 512], F32, tag="sC", name=f"sC_{b}_{h}")
            sloc = [sA[:, 0:512], sB[:, 0:384], sC[:, 0:256], sB[:, 384:512]]
            for r in range(NR):
                ln = (NR - r) * P
                kT = qk_sb[dlo:dlo + DH, jk,
                           b * T + r * P: b * T + (r + 1) * P]
                qTr = qk_sb[dlo:dlo + DH, jq, b * T + r * P:(b + 1) * T]
                # r==3 shares sB's bank with r==1: start=False so it does not
                # re-zero the 2KB zero-region holding r1's live data; its own
                # bytes are still pending-zero from r1's start.
                nc.tensor.matmul(sloc[r], kT, qTr, start=(r != 3), stop=True,
                                 skip_group_check=(r == 3))
                nc.scalar.activation(
                    out=pT[:, POFF[r]:POFF[r] + ln], in_=sloc[r],
                    func=mybir.ActivationFunctionType.Exp,
                    bias=kpmb_sb[:, b * NR + r: b * NR + r + 1])
                nc.vector.tensor_mul(
                    out=pT[:, POFF[r]:POFF[r] + P],
                    in0=pT[:, POFF[r]:POFF[r] + P],
                    in1=maskd_sb[:, r, :])

        def emit_v(n):
            for m in range(TOK // P):
                ps = ps_qk.tile([P, 512], F32, tag="ps", name=f"ps_v{n}_{m}")
                for k in range(CT):
                    nc.tensor.matmul(
                        ps[:], xk[k][:, m * P:(m + 1) * P], wv_sb[n][:, k, :],
                        start=(k == 0), stop=(k == CT - 1))
                nc.vector.tensor_copy(
                    out=v_sb[:, m, 8 * n:8 * n + 8, 0:DH],
                    in_=ps[:].rearrange("p (h d) -> p h d", d=DH))

        lrowp = {}

        def emit_av_pair(b, p):
            half = p // 4
            if (b, half) not in lrowp:
                lrowp[(b, half)] = lnx_pool.tile(
                    [DH + 1, 8, 512], DT, tag="lrowp",
                    name=f"lrowp_{b}_{half}")
            lt = lrowp[(b, half)]
            for kap in range(2):
                h = 2 * p + kap
                pT = pt_tiles.pop((b, h))
                po = ps_o.tile([P, 512], F32, tag="po", name=f"po_{b}_{h}")
                for r in range(NR):
                    ln = (NR - r) * P
                    nc.tensor.matmul(po[0:DH + 1, r * P:],
                                     v_sb[:, b * NR + r, h, :],
                                     pT[:, POFF[r]:POFF[r] + ln],
                                     start=(r == 0), stop=(r == NR - 1))
                # park the softmax row-sum (ones column) for this half
                nc.scalar.copy(out=lt[DH:DH + 1, (p % 4) * 2 + kap, :],
                               in_=po[DH:DH + 1, :])
                # drain attention output unnormalized (PSUM freed fast)
                if kap == 0:
                    nc.vector.tensor_copy(out=ao_b[b][0:DH, p, :],
                                          in_=po[0:DH, :])
                else:
                    ao_st = aost_pool.tile([DH, 512], DT, tag="aost",
                                           name=f"aost_{b}_{p}")
                    nc.vector.tensor_copy(out=ao_st[:], in_=po[0:DH, :])
                    nc.sync.dma_start(out=ao_b[b][DH:P, p, :], in_=ao_st[:])

        def emit_norm_half(b, half):
            """Gather the half's 8 row-sum rows (one SBUF->SBUF DMA), take
            1/l with an elementwise-reshaped [64,64] reciprocal (64 lanes
            instead of 8 -> ~5x faster), bounce through DRAM with stride-0
            broadcast DMAs and normalize the four ao c-tiles in place."""
            lt = lrowp.pop((b, half))
            nc.sync.dma_start(out=lgL[b][:], in_=lt[DH:DH + 1, :, :])
            liF = li_pool.tile([DH, DH], F32, tag="liF",
                               name=f"liF_{b}_{half}")
            nc.vector.reciprocal(out=liF[:], in_=lgL[b][:])
            lpd = li_pool.tile([DH, DH], DT, tag="lpd",
                               name=f"lpd_{b}_{half}")
            nc.scalar.copy(out=lpd[:], in_=liF[:])
            hs = slice(half * 8, half * 8 + 8)
            nc.sync.dma_start(out=linv[b, hs], in_=lpd[:])
            lf4 = lfsb_pool.tile([P, 4, 512], DT, tag="lf4",
                                 name=f"lf4_{b}_{half}")
            base = linv.offset + (b * H + half * 8) * T
            for kap in range(2):
                src_ap = bass.AP(tensor=linv.tensor, offset=base + kap * T,
                                 ap=[[0, DH], [2 * T, 4], [1, T]])
                nc.sync.dma_start(out=lf4[kap * DH:(kap + 1) * DH, :, :],
                                  in_=src_ap)
            for pp in range(4):
                p = half * 4 + pp
                nc.vector.tensor_mul(out=ao_b[b][:, p, :],
                                     in0=ao_b[b][:, p, :],
                                     in1=lf4[:, pp, :])

        def yproj_chunk(b, i):
            n, m = i // NR, i % NR
            ps = ps_qk.tile([P, 512], F32, tag="ps", name=f"ps_y{b}_{n}_{m}")
            for k in range(CT):
                nc.tensor.matmul(ps[:], ao_b[b][:, k, m * P:(m + 1) * P],
                                 wo_sb[n][:, k, :],
                                 start=(k == 0), stop=(k == CT - 1))
            y = y_pool.tile([P, 512], DT, tag="y")
            nc.vector.tensor_add(out=y[:], in0=ps[:],
                                 in1=bias_sb[:, n * 512:(n + 1) * 512])
            nc.scalar.dma_start(
                out=out[b * T + m * P: b * T + (m + 1) * P,
                        n * 512:(n + 1) * 512],
                in_=y[:])

        # --- main interleaved schedule ---
        for p in range(8):
            emit_qk(p, wq0 if p == 0 else None)
            emit_qk(8 + p)
            for b in range(B_LOC):
                for kap in range(2):
                    emit_scores(b, 2 * p + kap)
            if p == 0:
                nc.sync.dma_start(out=wv_sb[0][:], in_=wv[:, 0])
            elif p == 1:
                emit_v(0)
                nc.sync.dma_start(out=wv_sb[1][:], in_=wv[:, 1])
                for b in range(B_LOC):
                    emit_av_pair(b, 0)
            elif p == 2:
                for b in range(B_LOC):
                    emit_av_pair(b, 1)
            elif p == 3:
                emit_v(1)
                for b in range(B_LOC):
                    emit_av_pair(b, 2)
            elif p == 4:
                for b in range(B_LOC):
                    emit_av_pair(b, 3)
                for n in range(2):
                    nc.sync.dma_start(out=wo_sb[n][:], in_=wo[:, n])
            elif p == 5:
                for b in range(B_LOC):
                    emit_av_pair(b, 4)
            elif p == 6:
                for b in range(B_LOC):
                    emit_norm_half(b, 0)
                for b in range(B_LOC):
                    emit_av_pair(b, 5)
            elif p == 7:
                for b in range(B_LOC):
                    emit_av_pair(b, 6)
        emit_av_pair(0, 7)
        emit_norm_half(0, 1)
        emit_av_pair(1, 7)
        emit_norm_half(1, 1)
        for b in range(B_LOC):
            for i in range(2 * NR):
                yproj_chunk(b, i)
        if dbg is not None:
            nc.sync.dma_start(out=dbg["dbg_qk"][:], in_=qk_sb[:])

            nc.sync.dma_start(out=dbg["dbg_v"][:], in_=v_sb[:])
            for b in range(B_LOC):
                nc.sync.dma_start(out=dbg["dbg_ao"][b], in_=ao_b[b][:])


_NC_CACHE = None


def _get_nc():
    global _NC_CACHE
    if _NC_CACHE is None:
        _NC_CACHE = _build_nc()
    return _NC_CACHE


def _prep_core_inputs(x, mask, key_padding_mask, w_qkv, w_out, b_out):
    """Host-side sharding + layout prep. Returns list of per-core in_maps."""
    x = np.asarray(x, dtype=np.float32)
    mask = np.asarray(mask)
    kpm = np.asarray(key_padding_mask)
    w_qkv = np.asarray(w_qkv, dtype=np.float32)
    w_out = np.asarray(w_out, dtype=np.float32)
    b_out = np.asarray(b_out, dtype=np.float32)

    scale = 1.0 / math.sqrt(DH)
    w2 = w_qkv[:2 * C].copy()
    w2[:C] *= scale  # fold 1/sqrt(dh) into the Q weights
    # [j, p, k, f]: contiguous 2KB/partition DMA per j-tile
    wqk_r = np.ascontiguousarray(
        w2.reshape(16, P, CT, P).transpose(0, 3, 2, 1).astype(np.float16))
    # wv/wo as [p, n, k, f]: contiguous per-partition lines
    wv_r = np.ascontiguousarray(
        w_qkv[2 * C:].T.reshape(CT, P, 2, 512).transpose(1, 2, 0, 3)
        .astype(np.float16))
    wo_r = np.ascontiguousarray(
        w_out.T.reshape(CT, P, 2, 512).transpose(1, 2, 0, 3)
        .astype(np.float16))

    # The kernel exploits the causal structure: it only applies mask values
    # inside the diagonal 128x128 blocks and zero-fills fully-masked blocks.
    exp_tril = np.tril(np.ones((T, T), dtype=mask.dtype))
    assert np.array_equal(mask, exp_tril), "kernel assumes causal tril mask"
    maskTf = mask.T.astype(np.float16)  # [kt, qt]
    maskd = np.stack([maskTf[r * P:(r + 1) * P, r * P:(r + 1) * P]
                      for r in range(NR)])  # [NR, P, P]

    in_maps = []
    for i in range(N_CORES):
        xs = x[i * B_LOC:(i + 1) * B_LOC]      # [B_LOC, T, C]
        xT = np.ascontiguousarray(xs.reshape(TOK, C).T.astype(np.float16))
        kb = np.where(kpm[i * B_LOC:(i + 1) * B_LOC], -1e30,
                      0.0).astype(np.float32)  # [B_LOC, T]
        in_maps.append({
            "xT": xT,
            "wqk": wqk_r,
            "wv": wv_r,
            "wo": wo_r,
            "maskd": np.ascontiguousarray(maskd),
            "kpmb": kb,
            "bias": b_out,
        })
    return in_maps


def kernel(x, mask, key_padding_mask, w_qkv, w_out, b_out, _trace=False,
           _tmpdir=None):
    nc = _get_nc()
    in_maps = _prep_core_inputs(x, mask, key_padding_mask, w_qkv, w_out, b_out)
    res = run_bass_kernel_spmd(nc, in_maps, list(range(N_CORES)),
                               trace=_trace, tmpdir=_tmpdir)
    outs = [np.asarray(res.results[i]["out"], dtype=np.float32)
            .reshape(B_LOC, T, C) for i in range(N_CORES)]
    full = np.concatenate(outs, axis=0)
    kernel._last_exec_time_ns = res.exec_time_ns
    return full



# revision 4
# speedup vs baseline: 1.0138x; 1.0138x over previous
"""Multi-head self-attention (B=16,T=512,C=1024,H=16) on 8 NeuronCores.

Strategy: data-parallel over batch (2 batches/core), no collectives.
Schedule keeps the PE dense (HAM stays warm at 2.4GHz) and hides the
scalar-engine exp (the softmax) under the projection matmuls:

  pair p: QK-project head pair p -> scores + exp + mask for 4 (b,h) chains
  V projection + AV + out-projection woven between pairs so every engine
  always has work and nothing big sits at the tail.

Layouts avoid on-device transposes (same tricks as the ancestor kernel):
  - QK projection emits [f, tok]; scores are computed transposed
    sT[kt, qt]; softmax sums come from a ones-column appended to v.
  - AV drains move all 65 PSUM rows (64 ao + 1 l-row) to a staging tile;
    SBUF->SBUF DMAs then scatter the ao rows into place and the l-row
    straight into the [64,64] reciprocal layout (lgq), keeping the l
    bookkeeping off the ACT engine entirely.
  - softmax 1/l: reciprocal on the [64,64] tile (64 DVE lanes), cast,
    DRAM write + stride-0 broadcast read-back, one fused [128,4,512]
    normalize mul per (b,half); b=0 chains ride the sync DMA queue and
    b=1 the gpsimd queue so the two tails overlap.
  - out-projection: the first 6 chunks of b=0 run split-k: k0..3 (heads
    0..7, normalized early) accumulate while the half-1 norm chains
    drain, then k4..7 finish. The PE never idles over the norm latency.
Engine roles: PE matmuls only; scalar softmax exp; vector drains, diag
masks, reciprocal, normalize; sync + scalar + gpsimd DMA queues split
so x/weights, the l-chain and out-writes never serialize behind each
other. PSUM: 2 banks QK/V/yproj + 3 scores (r1/r3 share a bank via a
start=False accumulate into the still-pending zero-region) + 3 AV.
"""

import math
from contextlib import ExitStack

import numpy as np

import concourse.bass as bass
import concourse.mybir as mybir
import concourse.tile as tile
from concourse import bacc
from concourse.bass_utils import run_bass_kernel_spmd

N_CORES = 8
B, T, C = 16, 512, 1024
H = 16
DH = C // H  # 64
B_LOC = B // N_CORES  # 2
TOK = B_LOC * T  # 1024 tokens per core
P = 128
CT = C // P  # 8 contraction tiles
NR = T // P  # 4 kt blocks
DT = mybir.dt.float16
F32 = mybir.dt.float32

# compact pT column offsets per kt-block r (lengths 512,384,256,128)
POFF = [0, 512, 896, 1152]
PTW = 1280

N_ST1 = 6  # b=0 out-proj chunks run as split-k (bridge the norm latency)


def _build_nc():
    nc = bacc.Bacc("TRN2", target_bir_lowering=False, debug=False,
                   num_devices=N_CORES)

    xT = nc.dram_tensor("xT", [C, TOK], DT, kind="ExternalInput").ap()
    wqk = nc.dram_tensor("wqk", [16, P, CT, P], DT, kind="ExternalInput").ap()
    wv = nc.dram_tensor("wv", [P, 2, CT, 512], DT, kind="ExternalInput").ap()
    wo = nc.dram_tensor("wo", [P, 2, CT, 512], DT, kind="ExternalInput").ap()
    maskd = nc.dram_tensor("maskd", [NR, P, P], DT,
                           kind="ExternalInput").ap()
    kpmb = nc.dram_tensor("kpmb", [B_LOC, T], F32, kind="ExternalInput").ap()
    bias = nc.dram_tensor("bias", [C], F32, kind="ExternalInput").ap()
    # two copies: each DMA queue reads back the copy it wrote (queue FIFO
    # is the only ordering guarantee for raw DRAM tensors)
    linv = nc.dram_tensor("linv", [B_LOC, 2, H, T], DT).ap()
    out = nc.dram_tensor("out", [TOK, C], DT, kind="ExternalOutput").ap()

    with tile.TileContext(nc) as tc:
        _emit(nc, tc, xT, wqk, wv, wo, maskd, kpmb, bias, linv, out)

    nc.compile()
    return nc


def _emit(nc, tc, xT, wqk, wv, wo, maskd, kpmb, bias, linv, out):
    ctx = ExitStack()
    with ctx:
        singles = ctx.enter_context(tc.tile_pool(name="singles", bufs=1))
        ps_qk = ctx.enter_context(tc.tile_pool(name="ps_qk", bufs=2,
                                               space="PSUM"))
        ps_s = ctx.enter_context(tc.tile_pool(name="ps_s", bufs=1,
                                              space="PSUM"))
        ps_o = ctx.enter_context(tc.tile_pool(name="ps_o", bufs=3,
                                              space="PSUM"))
        wq_pool = ctx.enter_context(tc.tile_pool(name="wq", bufs=4))
        pt_pool = ctx.enter_context(tc.tile_pool(name="pt", bufs=11))
        li_pool = ctx.enter_context(tc.tile_pool(name="li", bufs=2))
        lfsb_pool = ctx.enter_context(tc.tile_pool(name="lfsb", bufs=2))
        aost_pool = ctx.enter_context(tc.tile_pool(name="aost", bufs=4))
        psv_pool = ctx.enter_context(tc.tile_pool(name="psv", bufs=N_ST1))
        y_pool = ctx.enter_context(tc.tile_pool(name="y", bufs=3))

        # --- persistent SBUF tensors ---
        qk_sb = singles.tile([P, 16, TOK], DT)             # 32 KB/part
        v_sb = singles.tile([P, TOK // P, H, DH + 1], DT)  # 16.6 KB/part
        ao_b = [singles.tile([P, CT, T], DT, name=f"ao_b{b}")
                for b in range(B_LOC)]                     # 2x 8 KB/part
        wv_sb = [singles.tile([P, CT, 512], DT, name=f"wv_{n}")
                 for n in range(2)]                        # 16 KB/part
        wo_sb = [singles.tile([P, CT, 512], DT, name=f"wo_{n}")
                 for n in range(2)]                        # 16 KB/part

        # per-(b,half) 1/l gather targets: row 8*hh+s = head hh, qt seg s
        lgq = {(b, hf): singles.tile([DH, DH], DT, name=f"lgq{b}_{hf}")
               for b in range(B_LOC) for hf in range(2)}
        bias_sb = singles.tile([P, C], F32)                # 4 KB/part
        maskd_sb = singles.tile([P, NR, P], DT)            # 1 KB/part
        kpmb_sb = singles.tile([P, B_LOC * NR], F32)
        xk = [singles.tile([P, TOK], DT, name=f"x_{k}") for k in range(CT)]

        # --- prologue DMAs ---
        # First matmul needs wq0[:,0,:] + xk0[:,0:512]; split the big tiles
        # into halves and fan them across all three queues so the PE can
        # start ~4us earlier and is never gated on a single 256KB transfer.
        wq0 = wq_pool.tile([P, CT, P], DT, tag="wq", name="wq_0")
        nc.sync.dma_start(out=wq0[:, 0:4, :], in_=wqk[0, :, 0:4, :])
        nc.scalar.dma_start(out=xk[0][:, 0:512], in_=xT[0:P, 0:512])
        nc.gpsimd.dma_start(out=wq0[:, 4:8, :], in_=wqk[0, :, 4:8, :])
        # first halves of every xk (the tt=0 chain), spread over queues
        nc.sync.dma_start(out=xk[1][:, 0:512], in_=xT[P:2 * P, 0:512])
        nc.scalar.dma_start(out=xk[2][:, 0:512],
                            in_=xT[2 * P:3 * P, 0:512])
        nc.gpsimd.dma_start(out=xk[3][:, 0:512],
                            in_=xT[3 * P:4 * P, 0:512])
        nc.sync.dma_start(out=xk[4][:, 0:512], in_=xT[4 * P:5 * P, 0:512])
        nc.scalar.dma_start(out=xk[5][:, 0:512],
                            in_=xT[5 * P:6 * P, 0:512])
        nc.gpsimd.dma_start(out=xk[6][:, 0:512],
                            in_=xT[6 * P:7 * P, 0:512])
        nc.sync.dma_start(out=xk[7][:, 0:512], in_=xT[7 * P:8 * P, 0:512])
        # second halves (the tt=1 chain)
        for k in range(CT):
            eng = (nc.scalar, nc.gpsimd, nc.sync)[k % 3]
            eng.dma_start(out=xk[k][:, 512:1024],
                          in_=xT[k * P:(k + 1) * P, 512:1024])
        nc.scalar.dma_start(out=maskd_sb[:],
                            in_=maskd.rearrange("r p q -> p r q"))
        nc.scalar.dma_start(out=kpmb_sb[:],
                            in_=kpmb.rearrange("b (r p) -> p (b r)", p=P))
        nc.vector.memset(v_sb[:, :, :, DH:DH + 1], 1.0)

        def emit_qk(j, wq_tile=None):
            if wq_tile is None:
                wq_tile = wq_pool.tile([P, CT, P], DT, tag="wq",
                                       name=f"wq_{j}")
                nc.sync.dma_start(out=wq_tile[:], in_=wqk[j])
            for tt in range(2):
                ps = ps_qk.tile([P, 512], F32, tag="ps", name=f"ps_qk{j}_{tt}")
                for k in range(CT):
                    nc.tensor.matmul(ps[:], wq_tile[:, k, :],
                                     xk[k][:, tt * 512:(tt + 1) * 512],
                                     start=(k == 0), stop=(k == CT - 1))
                nc.vector.tensor_copy(
                    out=qk_sb[:, j, tt * 512:(tt + 1) * 512], in_=ps[:])

        pt_tiles = {}

        def emit_scores(b, h):
            p = h // 2
            dlo = DH * (h % 2)
            jq, jk = p, 8 + p
            pT = pt_pool.tile([P, PTW], DT, tag="pT", name=f"pT_{b}_{h}")
            pt_tiles[(b, h)] = pT
            sA = ps_s.tile([P, 512], F32, tag="sA", name=f"sA_{b}_{h}")
            sB = ps_s.tile([P, 512], F32, tag="sB", name=f"sB_{b}_{h}")
            sC = ps_s.tile([P, 512], F32, tag="sC", name=f"sC_{b}_{h}")
            sloc = [sA[:, 0:512], sB[:, 0:384], sC[:, 0:256], sB[:, 384:512]]
            for r in range(NR):
                ln = (NR - r) * P
                kT = qk_sb[dlo:dlo + DH, jk,
                           b * T + r * P: b * T + (r + 1) * P]
                qTr = qk_sb[dlo:dlo + DH, jq, b * T + r * P:(b + 1) * T]
                # r==3 shares sB's bank with r==1: start=False so it does not
                # re-zero the 2KB zero-region holding r1's live data; its own
                # bytes are still pending-zero from r1's start.
                nc.tensor.matmul(sloc[r], kT, qTr, start=(r != 3), stop=True,
                                 skip_group_check=(r == 3))
                nc.scalar.activation(
                    out=pT[:, POFF[r]:POFF[r] + ln], in_=sloc[r],
                    func=mybir.ActivationFunctionType.Exp,
                    bias=kpmb_sb[:, b * NR + r: b * NR + r + 1])
                nc.vector.tensor_mul(
                    out=pT[:, POFF[r]:POFF[r] + P],
                    in0=pT[:, POFF[r]:POFF[r] + P],
                    in1=maskd_sb[:, r, :])

        def emit_v(n):
            for m in range(TOK // P):
                ps = ps_qk.tile([P, 512], F32, tag="ps", name=f"ps_v{n}_{m}")
                for k in range(CT):
                    nc.tensor.matmul(
                        ps[:], xk[k][:, m * P:(m + 1) * P], wv_sb[n][:, k, :],
                        start=(k == 0), stop=(k == CT - 1))
                nc.vector.tensor_copy(
                    out=v_sb[:, m, 8 * n:8 * n + 8, 0:DH],
                    in_=ps[:].rearrange("p (h d) -> p h d", d=DH))

        def emit_av_pair(b, p):
            half = p // 4
            for kap in range(2):
                h = 2 * p + kap
                pT = pt_tiles.pop((b, h))
                po = ps_o.tile([P, 512], F32, tag="po", name=f"po_{b}_{h}")
                for r in range(NR):
                    ln = (NR - r) * P
                    nc.tensor.matmul(po[0:DH + 1, r * P:],
                                     v_sb[:, b * NR + r, h, :],
                                     pT[:, POFF[r]:POFF[r] + ln],
                                     start=(r == 0), stop=(r == NR - 1))
                # drain all 65 rows (64 ao + the ones-column l row) in one
                # DVE op, then scatter by DMA: ao rows into ao_b, the l row
                # straight into the [64,64] reciprocal layout.
                st = aost_pool.tile([DH + 1, 512], DT, tag="aost",
                                    name=f"aost_{b}_{h}")
                nc.vector.tensor_copy(out=st[:], in_=po[0:DH + 1, :])
                hh = (p % 4) * 2 + kap
                lg = lgq[(b, half)]
                nc.sync.dma_start(out=lg[8 * hh:8 * hh + 8, :],
                                  in_=st[DH:DH + 1, :])
                eng = nc.gpsimd if (b + kap) % 2 else nc.sync
                eng.dma_start(
                    out=ao_b[b][kap * DH:(kap + 1) * DH, p, :],
                    in_=st[0:DH, :])

        def emit_norm_half(b, half):
            """1/l for 8 heads: reciprocal on the [64,64] gather (64 DVE
            lanes), cast, DRAM bounce with stride-0 broadcast reads, one
            fused [128,4,512] normalize mul. b=0 uses the sync queue, b=1
            gpsimd, so the two tail chains overlap."""
            qa = nc.sync if b == 0 else nc.gpsimd
            qb = nc.gpsimd if b == 0 else nc.sync
            lg = lgq[(b, half)]
            liF = li_pool.tile([DH, DH], F32, tag="liF",
                               name=f"liF_{b}_{half}")
            nc.vector.reciprocal(out=liF[:], in_=lg[:])
            lpd = li_pool.tile([DH, DH], DT, tag="lpd",
                               name=f"lpd_{b}_{half}")
            nc.vector.tensor_copy(out=lpd[:], in_=liF[:])
            hs = slice(half * 8, half * 8 + 8)
            qa.dma_start(out=linv[b, 0, hs], in_=lpd[:])
            qb.dma_start(out=linv[b, 1, hs], in_=lpd[:])
            lf4 = lfsb_pool.tile([P, 4, 512], DT, tag="lf4",
                                 name=f"lf4_{b}_{half}")
            for kap in range(2):
                base = linv.offset + ((b * 2 + kap) * H + half * 8) * T
                src_ap = bass.AP(tensor=linv.tensor, offset=base + kap * T,
                                 ap=[[0, DH], [2 * T, 4], [1, T]])
                (qa if kap == 0 else qb).dma_start(
                    out=lf4[kap * DH:(kap + 1) * DH, :, :], in_=src_ap)
            nc.vector.tensor_mul(
                out=ao_b[b][:, half * 4:(half + 1) * 4, :],
                in0=ao_b[b][:, half * 4:(half + 1) * 4, :],
                in1=lf4[:])

        out_qs = [nc.scalar, nc.sync, nc.gpsimd]

        def yproj_chunk(b, i, qi=[0]):
            n, m = i // NR, i % NR
            ps = ps_qk.tile([P, 512], F32, tag="ps", name=f"ps_y{b}_{n}_{m}")
            for k in range(CT):
                nc.tensor.matmul(ps[:], ao_b[b][:, k, m * P:(m + 1) * P],
                                 wo_sb[n][:, k, :],
                                 start=(k == 0), stop=(k == CT - 1))
            y = y_pool.tile([P, 512], DT, tag="y")
            nc.vector.tensor_add(out=y[:], in0=ps[:],
                                 in1=bias_sb[:, n * 512:(n + 1) * 512])
            out_qs[qi[0] % 3].dma_start(
                out=out[b * T + m * P: b * T + (m + 1) * P,
                        n * 512:(n + 1) * 512],
                in_=y[:])
            qi[0] += 1

        psv_tiles = {}

        def yproj_stage1(b, i):
            """k0..3 partial (heads 0..7: normalized early) + bias, parked
            in SBUF; bridges the PE over the half-1 norm chains."""
            n, m = i // NR, i % NR
            ps = ps_qk.tile([P, 512], F32, tag="ps", name=f"ps_y1{b}_{n}_{m}")
            for k in range(4):
                nc.tensor.matmul(ps[:], ao_b[b][:, k, m * P:(m + 1) * P],
                                 wo_sb[n][:, k, :],
                                 start=(k == 0), stop=(k == 3))
            sv = psv_pool.tile([P, 512], DT, tag="psv", name=f"psv{b}_{i}")
            psv_tiles[(b, i)] = sv
            nc.vector.tensor_add(out=sv[:], in0=ps[:],
                                 in1=bias_sb[:, n * 512:(n + 1) * 512])

        def yproj_stage2(b, i, qi=[0]):
            n, m = i // NR, i % NR
            ps = ps_qk.tile([P, 512], F32, tag="ps", name=f"ps_y2{b}_{n}_{m}")
            for k in range(4, CT):
                nc.tensor.matmul(ps[:], ao_b[b][:, k, m * P:(m + 1) * P],
                                 wo_sb[n][:, k, :],
                                 start=(k == 4), stop=(k == CT - 1))
            y = y_pool.tile([P, 512], DT, tag="y")
            nc.vector.tensor_add(out=y[:], in0=ps[:],
                                 in1=psv_tiles.pop((b, i))[:])
            out_qs[qi[0] % 3].dma_start(
                out=out[b * T + m * P: b * T + (m + 1) * P,
                        n * 512:(n + 1) * 512],
                in_=y[:])
            qi[0] += 1

        # --- main interleaved schedule ---
        for p in range(8):
            emit_qk(p, wq0 if p == 0 else None)
            emit_qk(8 + p)
            for b in range(B_LOC):
                for kap in range(2):
                    emit_scores(b, 2 * p + kap)
            if p == 0:
                nc.sync.dma_start(out=wv_sb[0][:], in_=wv[:, 0])
            elif p == 1:
                emit_v(0)
                nc.sync.dma_start(out=wv_sb[1][:], in_=wv[:, 1])
                for b in range(B_LOC):
                    emit_av_pair(b, 0)
            elif p == 2:
                for b in range(B_LOC):
                    emit_av_pair(b, 1)
            elif p == 3:
                emit_v(1)
                for b in range(B_LOC):
                    emit_av_pair(b, 2)
            elif p == 4:
                for b in range(B_LOC):
                    emit_av_pair(b, 3)
                for n in range(2):
                    nc.sync.dma_start(out=wo_sb[n][:], in_=wo[:, n])
                bias_bcast = bass.AP(tensor=bias.tensor, offset=bias.offset,
                                     ap=[[0, P], *bias.ap])
                nc.gpsimd.dma_start(out=bias_sb[:], in_=bias_bcast)
            elif p == 5:
                for b in range(B_LOC):
                    emit_av_pair(b, 4)
            elif p == 6:
                for b in range(B_LOC):
                    emit_norm_half(b, 0)
                for b in range(B_LOC):
                    emit_av_pair(b, 5)
            elif p == 7:
                for b in range(B_LOC):
                    emit_av_pair(b, 6)
        emit_av_pair(0, 7)
        emit_norm_half(0, 1)
        emit_av_pair(1, 7)
        emit_norm_half(1, 1)
        # b=0: split-k chunks bridge the norm chains, then finish
        for i in range(N_ST1):
            yproj_stage1(0, i)
        for i in range(N_ST1):
            yproj_stage2(0, i)
        for i in range(N_ST1, 2 * NR):
            yproj_chunk(0, i)
        for i in range(2 * NR):
            yproj_chunk(1, i)


_NC_CACHE = None


def _get_nc():
    global _NC_CACHE
    if _NC_CACHE is None:
        _NC_CACHE = _build_nc()
    return _NC_CACHE


def _prep_core_inputs(x, mask, key_padding_mask, w_qkv, w_out, b_out):
    """Host-side sharding + layout prep. Returns list of per-core in_maps."""
    x = np.asarray(x, dtype=np.float32)
    mask = np.asarray(mask)
    kpm = np.asarray(key_padding_mask)
    w_qkv = np.asarray(w_qkv, dtype=np.float32)
    w_out = np.asarray(w_out, dtype=np.float32)
    b_out = np.asarray(b_out, dtype=np.float32)

    scale = 1.0 / math.sqrt(DH)
    w2 = w_qkv[:2 * C].copy()
    w2[:C] *= scale  # fold 1/sqrt(dh) into the Q weights
    # [j, p, k, f]: contiguous 2KB/partition DMA per j-tile
    wqk_r = np.ascontiguousarray(
        w2.reshape(16, P, CT, P).transpose(0, 3, 2, 1).astype(np.float16))
    # wv/wo as [p, n, k, f]: contiguous per-partition lines
    wv_r = np.ascontiguousarray(
        w_qkv[2 * C:].T.reshape(CT, P, 2, 512).transpose(1, 2, 0, 3)
        .astype(np.float16))
    wo_r = np.ascontiguousarray(
        w_out.T.reshape(CT, P, 2, 512).transpose(1, 2, 0, 3)
        .astype(np.float16))

    # The kernel exploits the causal structure: it only applies mask values
    # inside the diagonal 128x128 blocks and zero-fills fully-masked blocks.
    exp_tril = np.tril(np.ones((T, T), dtype=mask.dtype))
    assert np.array_equal(mask, exp_tril), "kernel assumes causal tril mask"
    maskTf = mask.T.astype(np.float16)  # [kt, qt]
    maskd = np.stack([maskTf[r * P:(r + 1) * P, r * P:(r + 1) * P]
                      for r in range(NR)])  # [NR, P, P]

    in_maps = []
    for i in range(N_CORES):
        xs = x[i * B_LOC:(i + 1) * B_LOC]      # [B_LOC, T, C]
        xT = np.ascontiguousarray(xs.reshape(TOK, C).T.astype(np.float16))
        kb = np.where(kpm[i * B_LOC:(i + 1) * B_LOC], -1e30,
                      0.0).astype(np.float32)  # [B_LOC, T]
        in_maps.append({
            "xT": xT,
            "wqk": wqk_r,
            "wv": wv_r,
            "wo": wo_r,
            "maskd": np.ascontiguousarray(maskd),
            "kpmb": kb,
            "bias": b_out,
        })
    return in_maps


def kernel(x, mask, key_padding_mask, w_qkv, w_out, b_out, _trace=False,
           _tmpdir=None):
    nc = _get_nc()
    in_maps = _prep_core_inputs(x, mask, key_padding_mask, w_qkv, w_out, b_out)
    res = run_bass_kernel_spmd(nc, in_maps, list(range(N_CORES)),
                               trace=_trace, tmpdir=_tmpdir)
    outs = [np.asarray(res.results[i]["out"], dtype=np.float32)
            .reshape(B_LOC, T, C) for i in range(N_CORES)]
    full = np.concatenate(outs, axis=0)
    kernel._last_exec_time_ns = res.exec_time_ns
    return full


# revision 20
# speedup vs baseline: 1.0140x; 1.0002x over previous
"""Multi-head self-attention (B=16,T=512,C=1024,H=16) on 8 NeuronCores.

Strategy: data-parallel over batch (2 batches/core), no collectives.
Schedule keeps the PE dense (HAM stays warm at 2.4GHz) and hides the
scalar-engine exp (the softmax) under the projection matmuls:

  pair p: QK-project head pair p -> scores + exp + mask for 4 (b,h) chains
  V projection + AV + out-projection woven between pairs so every engine
  always has work and nothing big sits at the tail.

Layouts avoid on-device transposes (same tricks as the ancestor kernel):
  - QK projection emits [f, tok]; scores are computed transposed
    sT[kt, qt]; softmax sums come from a ones-column appended to v.
  - AV drains move all 65 PSUM rows (64 ao + 1 l-row) to a staging tile;
    SBUF->SBUF DMAs then scatter the ao rows into place and the l-row
    straight into the [64,64] reciprocal layout (lgq), keeping the l
    bookkeeping off the ACT engine entirely.
  - softmax 1/l: reciprocal on the [64,64] tile (64 DVE lanes), cast,
    DRAM write + stride-0 broadcast read-back, one fused [128,4,512]
    normalize mul per (b,half); b=0 chains ride the sync DMA queue and
    b=1 the gpsimd queue so the two tails overlap.
  - out-projection: the first 6 chunks of b=0 run split-k: k0..3 (heads
    0..7, normalized early) accumulate while the half-1 norm chains
    drain, then k4..7 finish. The PE never idles over the norm latency.
Engine roles: PE matmuls only; scalar softmax exp; vector drains, diag
masks, reciprocal, normalize; sync + scalar + gpsimd DMA queues split
so x/weights, the l-chain and out-writes never serialize behind each
other. PSUM: 2 banks QK/V/yproj + 3 scores (r1/r3 share a bank via a
start=False accumulate into the still-pending zero-region) + 3 AV.
"""

import math
from contextlib import ExitStack

import numpy as np

import concourse.bass as bass
import concourse.mybir as mybir
import concourse.tile as tile
from concourse import bacc
from concourse.bass_utils import run_bass_kernel_spmd

N_CORES = 8
B, T, C = 16, 512, 1024
H = 16
DH = C // H  # 64
B_LOC = B // N_CORES  # 2
TOK = B_LOC * T  # 1024 tokens per core
P = 128
CT = C // P  # 8 contraction tiles
NR = T // P  # 4 kt blocks
DT = mybir.dt.float16
F32 = mybir.dt.float32

# compact pT column offsets per kt-block r (lengths 512,384,256,128)
POFF = [0, 512, 896, 1152]
PTW = 1280

N_ST1 = 6  # b=0 out-proj chunks run as split-k (bridge the norm latency)


def _build_nc():
    nc = bacc.Bacc("TRN2", target_bir_lowering=False, debug=False,
                   num_devices=N_CORES)

    xT = nc.dram_tensor("xT", [C, TOK], DT, kind="ExternalInput").ap()
    wqk = nc.dram_tensor("wqk", [16, P, CT, P], DT, kind="ExternalInput").ap()
    wv = nc.dram_tensor("wv", [P, 2, CT, 512], DT, kind="ExternalInput").ap()
    wo = nc.dram_tensor("wo", [P, 2, CT, 512], DT, kind="ExternalInput").ap()
    maskd = nc.dram_tensor("maskd", [NR, P, P], DT,
                           kind="ExternalInput").ap()
    kpmb = nc.dram_tensor("kpmb", [B_LOC, T], F32, kind="ExternalInput").ap()
    bias = nc.dram_tensor("bias", [C], F32, kind="ExternalInput").ap()
    sel2d = nc.dram_tensor("sel2d", [2, P], DT, kind="ExternalInput").ap()
    out = nc.dram_tensor("out", [TOK, C], DT, kind="ExternalOutput").ap()

    with tile.TileContext(nc) as tc:
        _emit(nc, tc, xT, wqk, wv, wo, maskd, kpmb, bias, sel2d, out)

    nc.compile()
    return nc


def _emit(nc, tc, xT, wqk, wv, wo, maskd, kpmb, bias, sel2d, out):
    ctx = ExitStack()
    with ctx:
        singles = ctx.enter_context(tc.tile_pool(name="singles", bufs=1))
        ps_qk = ctx.enter_context(tc.tile_pool(name="ps_qk", bufs=2,
                                               space="PSUM"))
        ps_s = ctx.enter_context(tc.tile_pool(name="ps_s", bufs=1,
                                              space="PSUM"))
        ps_o = ctx.enter_context(tc.tile_pool(name="ps_o", bufs=3,
                                              space="PSUM"))
        wq_pool = ctx.enter_context(tc.tile_pool(name="wq", bufs=4))
        pt_pool = ctx.enter_context(tc.tile_pool(name="pt", bufs=11))
        li_pool = ctx.enter_context(tc.tile_pool(name="li", bufs=2))
        aost_pool = ctx.enter_context(tc.tile_pool(name="aost", bufs=4))
        psv_pool = ctx.enter_context(tc.tile_pool(name="psv", bufs=N_ST1))
        y_pool = ctx.enter_context(tc.tile_pool(name="y", bufs=3))

        # --- persistent SBUF tensors ---
        qk_sb = singles.tile([P, 16, TOK], DT)             # 32 KB/part
        v_sb = singles.tile([P, TOK // P, H, DH + 1], DT)  # 16.6 KB/part
        ao_b = [singles.tile([P, CT, T], DT, name=f"ao_b{b}")
                for b in range(B_LOC)]                     # 2x 8 KB/part
        wv_sb = [singles.tile([P, CT, 512], DT, name=f"wv_{n}")
                 for n in range(2)]                        # 16 KB/part
        wo_sb = [singles.tile([P, CT, 512], DT, name=f"wo_{n}")
                 for n in range(2)]                        # 16 KB/part

        # per-(b,half) 1/l gather targets: row 32*kap+8*pp+s = head 2*pp+kap,
        # qt segment s (kap-major so a plain reshape DMA yields head rows)
        lgq = {(b, hf): singles.tile([DH, DH], DT, name=f"lgq{b}_{hf}")
               for b in range(B_LOC) for hf in range(2)}
        # [2,128] selector: broadcast head-row kap across partitions 64*kap..
        sel2 = singles.tile([2, P], DT)
        nc.gpsimd.dma_start(out=sel2[:], in_=sel2d)
        bias_sb = singles.tile([P, C], F32)                # 4 KB/part
        maskd_sb = singles.tile([P, NR, P], DT)            # 1 KB/part
        kpmb_sb = singles.tile([P, B_LOC * NR], F32)
        xk = [singles.tile([P, TOK], DT, name=f"x_{k}") for k in range(CT)]

        # --- prologue DMAs ---
        # First matmul needs wq0[:,0,:] + xk0[:,0:512]; split the big tiles
        # into halves and fan them across all three queues so the PE can
        # start ~4us earlier and is never gated on a single 256KB transfer.
        wq0 = wq_pool.tile([P, CT, P], DT, tag="wq", name="wq_0")
        nc.sync.dma_start(out=wq0[:, 0:4, :], in_=wqk[0, :, 0:4, :])
        nc.scalar.dma_start(out=xk[0][:, 0:512], in_=xT[0:P, 0:512])
        nc.gpsimd.dma_start(out=wq0[:, 4:8, :], in_=wqk[0, :, 4:8, :])
        # first halves of every xk (the tt=0 chain), spread over queues
        nc.sync.dma_start(out=xk[1][:, 0:512], in_=xT[P:2 * P, 0:512])
        nc.scalar.dma_start(out=xk[2][:, 0:512],
                            in_=xT[2 * P:3 * P, 0:512])
        nc.gpsimd.dma_start(out=xk[3][:, 0:512],
                            in_=xT[3 * P:4 * P, 0:512])
        nc.sync.dma_start(out=xk[4][:, 0:512], in_=xT[4 * P:5 * P, 0:512])
        nc.scalar.dma_start(out=xk[5][:, 0:512],
                            in_=xT[5 * P:6 * P, 0:512])
        nc.gpsimd.dma_start(out=xk[6][:, 0:512],
                            in_=xT[6 * P:7 * P, 0:512])
        nc.sync.dma_start(out=xk[7][:, 0:512], in_=xT[7 * P:8 * P, 0:512])
        # second halves (the tt=1 chain)
        for k in range(CT):
            eng = (nc.scalar, nc.gpsimd, nc.sync)[k % 3]
            eng.dma_start(out=xk[k][:, 512:1024],
                          in_=xT[k * P:(k + 1) * P, 512:1024])
        nc.scalar.dma_start(out=maskd_sb[:],
                            in_=maskd.rearrange("r p q -> p r q"))
        nc.scalar.dma_start(out=kpmb_sb[:],
                            in_=kpmb.rearrange("b (r p) -> p (b r)", p=P))
        nc.vector.memset(v_sb[:, :, :, DH:DH + 1], 1.0)

        def emit_qk(j, wq_tile=None):
            if wq_tile is None:
                wq_tile = wq_pool.tile([P, CT, P], DT, tag="wq",
                                       name=f"wq_{j}")
                nc.sync.dma_start(out=wq_tile[:], in_=wqk[j])
            for tt in range(2):
                ps = ps_qk.tile([P, 512], F32, tag="ps", name=f"ps_qk{j}_{tt}")
                for k in range(CT):
                    nc.tensor.matmul(ps[:], wq_tile[:, k, :],
                                     xk[k][:, tt * 512:(tt + 1) * 512],
                                     start=(k == 0), stop=(k == CT - 1))
                nc.vector.tensor_copy(
                    out=qk_sb[:, j, tt * 512:(tt + 1) * 512], in_=ps[:])

        pt_tiles = {}

        def emit_scores(b, h):
            p = h // 2
            dlo = DH * (h % 2)
            jq, jk = p, 8 + p
            pT = pt_pool.tile([P, PTW], DT, tag="pT", name=f"pT_{b}_{h}")
            pt_tiles[(b, h)] = pT
            sA = ps_s.tile([P, 512], F32, tag="sA", name=f"sA_{b}_{h}")
            sB = ps_s.tile([P, 512], F32, tag="sB", name=f"sB_{b}_{h}")
            sC = ps_s.tile([P, 512], F32, tag="sC", name=f"sC_{b}_{h}")
            sloc = [sA[:, 0:512], sB[:, 0:384], sC[:, 0:256], sB[:, 384:512]]
            for r in range(NR):
                ln = (NR - r) * P
                kT = qk_sb[dlo:dlo + DH, jk,
                           b * T + r * P: b * T + (r + 1) * P]
                qTr = qk_sb[dlo:dlo + DH, jq, b * T + r * P:(b + 1) * T]
                # r==3 shares sB's bank with r==1: start=False so it does not
                # re-zero the 2KB zero-region holding r1's live data; its own
                # bytes are still pending-zero from r1's start.
                nc.tensor.matmul(sloc[r], kT, qTr, start=(r != 3), stop=True,
                                 skip_group_check=(r == 3))
                nc.scalar.activation(
                    out=pT[:, POFF[r]:POFF[r] + ln], in_=sloc[r],
                    func=mybir.ActivationFunctionType.Exp,
                    bias=kpmb_sb[:, b * NR + r: b * NR + r + 1])
                # diag mask on the (otherwise idle) Pool engine, off DVE
                nc.gpsimd.tensor_mul(
                    out=pT[:, POFF[r]:POFF[r] + P],
                    in0=pT[:, POFF[r]:POFF[r] + P],
                    in1=maskd_sb[:, r, :])

        def emit_v(n):
            for m in range(TOK // P):
                ps = ps_qk.tile([P, 512], F32, tag="ps", name=f"ps_v{n}_{m}")
                for k in range(CT):
                    nc.tensor.matmul(
                        ps[:], xk[k][:, m * P:(m + 1) * P], wv_sb[n][:, k, :],
                        start=(k == 0), stop=(k == CT - 1))
                nc.vector.tensor_copy(
                    out=v_sb[:, m, 8 * n:8 * n + 8, 0:DH],
                    in_=ps[:].rearrange("p (h d) -> p h d", d=DH))

        def emit_av_pair(b, p):
            half = p // 4
            for kap in range(2):
                h = 2 * p + kap
                pT = pt_tiles.pop((b, h))
                po = ps_o.tile([P, 512], F32, tag="po", name=f"po_{b}_{h}")
                for r in range(NR):
                    ln = (NR - r) * P
                    nc.tensor.matmul(po[0:DH + 1, r * P:],
                                     v_sb[:, b * NR + r, h, :],
                                     pT[:, POFF[r]:POFF[r] + ln],
                                     start=(r == 0), stop=(r == NR - 1))
                # drain all 65 rows (64 ao + the ones-column l row) in one
                # DVE op, then scatter by DMA: ao rows into ao_b, the l row
                # straight into the [64,64] reciprocal layout.
                st = aost_pool.tile([DH + 1, 512], DT, tag="aost",
                                    name=f"aost_{b}_{h}")
                nc.vector.tensor_copy(out=st[:], in_=po[0:DH + 1, :])
                rr = 32 * kap + 8 * (p % 4)
                lg = lgq[(b, half)]
                nc.sync.dma_start(out=lg[rr:rr + 8, :],
                                  in_=st[DH:DH + 1, :])
                eng = nc.gpsimd if (b + kap) % 2 else nc.sync
                eng.dma_start(
                    out=ao_b[b][kap * DH:(kap + 1) * DH, p, :],
                    in_=st[0:DH, :])

        def emit_norm_half(b, half):
            """1/l for 8 heads: reciprocal on the [64,64] gather (64 DVE
            lanes), cast, reshape-DMA into [2, 4, 512] head rows, then a
            K=2 selector matmul per pair broadcasts 1/l across the 128
            partitions in PSUM; the normalize mul reads it straight from
            there. No DRAM bounce, ~10KB of DMA instead of ~1MB."""
            qa = nc.sync if b == 0 else nc.gpsimd
            lg = lgq[(b, half)]
            liF = li_pool.tile([DH, DH], F32, tag="liF",
                               name=f"liF_{b}_{half}")
            nc.vector.reciprocal(out=liF[:], in_=lg[:])
            lpd = li_pool.tile([DH, DH], DT, tag="lpd",
                               name=f"lpd_{b}_{half}")
            nc.scalar.copy(out=lpd[:], in_=liF[:])
            lrow = li_pool.tile([2, 4, 512], DT, tag="lrow",
                                name=f"lrow_{b}_{half}")
            qa.dma_start(out=lrow[:], in_=lpd[:])
            for pp in range(4):
                lf = ps_o.tile([P, 512], F32, tag="po",
                               name=f"lf_{b}_{half}_{pp}")
                nc.tensor.matmul(lf[:], sel2[:], lrow[:, pp, :],
                                 start=True, stop=True)
                nc.vector.tensor_mul(
                    out=ao_b[b][:, half * 4 + pp, :],
                    in0=ao_b[b][:, half * 4 + pp, :],
                    in1=lf[:])

        out_qs = [nc.scalar, nc.sync, nc.gpsimd]

        def yproj_chunk(b, i, qi=[0]):
            n, m = i // NR, i % NR
            ps = ps_qk.tile([P, 512], F32, tag="ps", name=f"ps_y{b}_{n}_{m}")
            for k in range(CT):
                nc.tensor.matmul(ps[:], ao_b[b][:, k, m * P:(m + 1) * P],
                                 wo_sb[n][:, k, :],
                                 start=(k == 0), stop=(k == CT - 1))
            y = y_pool.tile([P, 512], DT, tag="y")
            nc.vector.tensor_add(out=y[:], in0=ps[:],
                                 in1=bias_sb[:, n * 512:(n + 1) * 512])
            out_qs[qi[0] % 3].dma_start(
                out=out[b * T + m * P: b * T + (m + 1) * P,
                        n * 512:(n + 1) * 512],
                in_=y[:])
            qi[0] += 1

        psv_tiles = {}

        def yproj_stage1(b, i):
            """k0..3 partial (heads 0..7: normalized early) + bias, parked
            in SBUF; bridges the PE over the half-1 norm chains."""
            n, m = i // NR, i % NR
            ps = ps_qk.tile([P, 512], F32, tag="ps", name=f"ps_y1{b}_{n}_{m}")
            for k in range(4):
                nc.tensor.matmul(ps[:], ao_b[b][:, k, m * P:(m + 1) * P],
                                 wo_sb[n][:, k, :],
                                 start=(k == 0), stop=(k == 3))
            sv = psv_pool.tile([P, 512], DT, tag="psv", name=f"psv{b}_{i}")
            psv_tiles[(b, i)] = sv
            nc.vector.tensor_add(out=sv[:], in0=ps[:],
                                 in1=bias_sb[:, n * 512:(n + 1) * 512])

        def yproj_stage2(b, i, qi=[0]):
            n, m = i // NR, i % NR
            ps = ps_qk.tile([P, 512], F32, tag="ps", name=f"ps_y2{b}_{n}_{m}")
            for k in range(4, CT):
                nc.tensor.matmul(ps[:], ao_b[b][:, k, m * P:(m + 1) * P],
                                 wo_sb[n][:, k, :],
                                 start=(k == 4), stop=(k == CT - 1))
            y = y_pool.tile([P, 512], DT, tag="y")
            nc.vector.tensor_add(out=y[:], in0=ps[:],
                                 in1=psv_tiles.pop((b, i))[:])
            out_qs[qi[0] % 3].dma_start(
                out=out[b * T + m * P: b * T + (m + 1) * P,
                        n * 512:(n + 1) * 512],
                in_=y[:])
            qi[0] += 1

        # --- main interleaved schedule ---
        for p in range(8):
            emit_qk(p, wq0 if p == 0 else None)
            emit_qk(8 + p)
            for b in range(B_LOC):
                for kap in range(2):
                    emit_scores(b, 2 * p + kap)
            if p == 0:
                nc.sync.dma_start(out=wv_sb[0][:], in_=wv[:, 0])
            elif p == 1:
                emit_v(0)
                nc.sync.dma_start(out=wv_sb[1][:], in_=wv[:, 1])
                for b in range(B_LOC):
                    emit_av_pair(b, 0)
            elif p == 2:
                for b in range(B_LOC):
                    emit_av_pair(b, 1)
            elif p == 3:
                emit_v(1)
                for b in range(B_LOC):
                    emit_av_pair(b, 2)
            elif p == 4:
                for b in range(B_LOC):
                    emit_av_pair(b, 3)
                for n in range(2):
                    nc.sync.dma_start(out=wo_sb[n][:], in_=wo[:, n])
                bias_bcast = bass.AP(tensor=bias.tensor, offset=bias.offset,
                                     ap=[[0, P], *bias.ap])
                nc.gpsimd.dma_start(out=bias_sb[:], in_=bias_bcast)
            elif p == 5:
                for b in range(B_LOC):
                    emit_av_pair(b, 4)
            elif p == 6:
                for b in range(B_LOC):
                    emit_norm_half(b, 0)
                for b in range(B_LOC):
                    emit_av_pair(b, 5)
            elif p == 7:
                for b in range(B_LOC):
                    emit_av_pair(b, 6)
        # tail: split-k b=0 chunks (k0..3 = heads 0..7, normalized early)
        # keep the PE busy while the half-1 norm chains resolve
        emit_av_pair(0, 7)
        yproj_stage1(0, 0)
        yproj_stage1(0, 1)
        emit_norm_half(0, 1)
        emit_av_pair(1, 7)
        yproj_stage1(0, 2)
        yproj_stage1(0, 3)
        emit_norm_half(1, 1)
        yproj_stage1(0, 4)
        yproj_stage1(0, 5)
        for i in range(N_ST1):
            yproj_stage2(0, i)
        for i in range(N_ST1, 2 * NR):
            yproj_chunk(0, i)
        for i in range(2 * NR):
            yproj_chunk(1, i)


_NC_CACHE = None


def _get_nc():
    global _NC_CACHE
    if _NC_CACHE is None:
        _NC_CACHE = _build_nc()
    return _NC_CACHE


def _prep_core_inputs(x, mask, key_padding_mask, w_qkv, w_out, b_out):
    """Host-side sharding + layout prep. Returns list of per-core in_maps."""
    x = np.asarray(x, dtype=np.float32)
    mask = np.asarray(mask)
    kpm = np.asarray(key_padding_mask)
    w_qkv = np.asarray(w_qkv, dtype=np.float32)
    w_out = np.asarray(w_out, dtype=np.float32)
    b_out = np.asarray(b_out, dtype=np.float32)

    scale = 1.0 / math.sqrt(DH)
    w2 = w_qkv[:2 * C].copy()
    w2[:C] *= scale  # fold 1/sqrt(dh) into the Q weights
    # [j, p, k, f]: contiguous 2KB/partition DMA per j-tile
    wqk_r = np.ascontiguousarray(
        w2.reshape(16, P, CT, P).transpose(0, 3, 2, 1).astype(np.float16))
    # wv/wo as [p, n, k, f]: contiguous per-partition lines
    wv_r = np.ascontiguousarray(
        w_qkv[2 * C:].T.reshape(CT, P, 2, 512).transpose(1, 2, 0, 3)
        .astype(np.float16))
    wo_r = np.ascontiguousarray(
        w_out.T.reshape(CT, P, 2, 512).transpose(1, 2, 0, 3)
        .astype(np.float16))

    # The kernel exploits the causal structure: it only applies mask values
    # inside the diagonal 128x128 blocks and zero-fills fully-masked blocks.
    exp_tril = np.tril(np.ones((T, T), dtype=mask.dtype))
    assert np.array_equal(mask, exp_tril), "kernel assumes causal tril mask"
    maskTf = mask.T.astype(np.float16)  # [kt, qt]
    maskd = np.stack([maskTf[r * P:(r + 1) * P, r * P:(r + 1) * P]
                      for r in range(NR)])  # [NR, P, P]

    # [2,128] selector for the 1/l partition broadcast matmul
    sel2_h = np.zeros((2, P), dtype=np.float16)
    sel2_h[0, 0:DH] = 1.0
    sel2_h[1, DH:P] = 1.0

    in_maps = []
    for i in range(N_CORES):
        xs = x[i * B_LOC:(i + 1) * B_LOC]      # [B_LOC, T, C]
        xT = np.ascontiguousarray(xs.reshape(TOK, C).T.astype(np.float16))
        kb = np.where(kpm[i * B_LOC:(i + 1) * B_LOC], -1e30,
                      0.0).astype(np.float32)  # [B_LOC, T]
        in_maps.append({
            "xT": xT,
            "wqk": wqk_r,
            "wv": wv_r,
            "wo": wo_r,
            "maskd": np.ascontiguousarray(maskd),
            "kpmb": kb,
            "bias": b_out,
            "sel2d": sel2_h,
        })
    return in_maps


def kernel(x, mask, key_padding_mask, w_qkv, w_out, b_out, _trace=False,
           _tmpdir=None):
    nc = _get_nc()
    in_maps = _prep_core_inputs(x, mask, key_padding_mask, w_qkv, w_out, b_out)
    res = run_bass_kernel_spmd(nc, in_maps, list(range(N_CORES)),
                               trace=_trace, tmpdir=_tmpdir)
    outs = [np.asarray(res.results[i]["out"], dtype=np.float32)
            .reshape(B_LOC, T, C) for i in range(N_CORES)]
    full = np.concatenate(outs, axis=0)
    kernel._last_exec_time_ns = res.exec_time_ns
    return full


# revision 25
# speedup vs baseline: 1.0359x; 1.0216x over previous
"""Multi-head self-attention (B=16,T=512,C=1024,H=16) on 8 NeuronCores.

Strategy: data-parallel over batch (2 batches/core), no collectives.
Schedule keeps the PE dense (HAM stays warm at 2.4GHz) and hides the
scalar-engine exp (the softmax) under the projection matmuls:

  pair p: QK-project head pair p -> scores + exp + mask for 4 (b,h) chains
  V projection + AV + out-projection woven between pairs so every engine
  always has work and nothing big sits at the tail.

Layouts avoid on-device transposes (same tricks as the ancestor kernel):
  - QK projection emits [f, tok]; scores are computed transposed
    sT[kt, qt]; softmax sums come from a ones-column appended to v.
  - AV drains move all 65 PSUM rows (64 ao + 1 l-row) to a staging tile;
    SBUF->SBUF DMAs then scatter the ao rows into place and the l-row
    straight into the [64,64] reciprocal layout (lgq), keeping the l
    bookkeeping off the ACT engine entirely.
  - softmax 1/l: reciprocal on the [64,64] tile (64 DVE lanes), cast,
    DRAM write + stride-0 broadcast read-back, one fused [128,4,512]
    normalize mul per (b,half); b=0 chains ride the sync DMA queue and
    b=1 the gpsimd queue so the two tails overlap.
  - out-projection: the first 6 chunks of b=0 run split-k: k0..3 (heads
    0..7, normalized early) accumulate while the half-1 norm chains
    drain, then k4..7 finish. The PE never idles over the norm latency.
Engine roles: PE matmuls only; scalar softmax exp; vector drains, diag
masks, reciprocal, normalize; sync + scalar + gpsimd DMA queues split
so x/weights, the l-chain and out-writes never serialize behind each
other. PSUM: 2 banks QK/V/yproj + 3 scores (r1/r3 share a bank via a
start=False accumulate into the still-pending zero-region) + 3 AV.
"""

import math
from contextlib import ExitStack

import numpy as np

import concourse.bass as bass
import concourse.mybir as mybir
import concourse.tile as tile
from concourse import bacc
from concourse.bass_utils import run_bass_kernel_spmd

N_CORES = 8
B, T, C = 16, 512, 1024
H = 16
DH = C // H  # 64
B_LOC = B // N_CORES  # 2
TOK = B_LOC * T  # 1024 tokens per core
P = 128
CT = C // P  # 8 contraction tiles
NR = T // P  # 4 kt blocks
DT = mybir.dt.float16
F32 = mybir.dt.float32

# compact pT column offsets per kt-block r (lengths 512,384,256,128)
POFF = [0, 512, 896, 1152]
PTW = 1280

N_ST1 = 6  # b=0 out-proj chunks run as split-k (bridge the norm latency)


def _build_nc():
    nc = bacc.Bacc("TRN2", target_bir_lowering=False, debug=False,
                   num_devices=N_CORES)

    xT = nc.dram_tensor("xT", [C, TOK], DT, kind="ExternalInput").ap()
    wqk = nc.dram_tensor("wqk", [16, P, CT, P], DT, kind="ExternalInput").ap()
    wv = nc.dram_tensor("wv", [P, 2, CT, 512], DT, kind="ExternalInput").ap()
    wo = nc.dram_tensor("wo", [P, 2, CT, 512], DT, kind="ExternalInput").ap()
    maskd = nc.dram_tensor("maskd", [NR, P, P], DT,
                           kind="ExternalInput").ap()
    kpmb = nc.dram_tensor("kpmb", [B_LOC, T], F32, kind="ExternalInput").ap()
    bias = nc.dram_tensor("bias", [C], F32, kind="ExternalInput").ap()
    sel2d = nc.dram_tensor("sel2d", [2, P], DT, kind="ExternalInput").ap()
    out = nc.dram_tensor("out", [TOK, C], DT, kind="ExternalOutput").ap()

    with tile.TileContext(nc) as tc:
        _emit(nc, tc, xT, wqk, wv, wo, maskd, kpmb, bias, sel2d, out)

    nc.compile()
    return nc


def _emit(nc, tc, xT, wqk, wv, wo, maskd, kpmb, bias, sel2d, out):
    ctx = ExitStack()
    with ctx:
        singles = ctx.enter_context(tc.tile_pool(name="singles", bufs=1))
        ps_qk = ctx.enter_context(tc.tile_pool(name="ps_qk", bufs=2,
                                               space="PSUM"))
        ps_s = ctx.enter_context(tc.tile_pool(name="ps_s", bufs=1,
                                              space="PSUM"))
        ps_o = ctx.enter_context(tc.tile_pool(name="ps_o", bufs=3,
                                              space="PSUM"))
        wq_pool = ctx.enter_context(tc.tile_pool(name="wq", bufs=4))
        pt_pool = ctx.enter_context(tc.tile_pool(name="pt", bufs=11))
        li_pool = ctx.enter_context(tc.tile_pool(name="li", bufs=2))
        aost_pool = ctx.enter_context(tc.tile_pool(name="aost", bufs=4))
        psv_pool = ctx.enter_context(tc.tile_pool(name="psv", bufs=N_ST1))
        y_pool = ctx.enter_context(tc.tile_pool(name="y", bufs=3))

        # --- persistent SBUF tensors ---
        qk_sb = singles.tile([P, 16, TOK], DT)             # 32 KB/part
        v_sb = singles.tile([P, TOK // P, H, DH + 1], DT)  # 16.6 KB/part
        ao_b = [singles.tile([P, CT, T], DT, name=f"ao_b{b}")
                for b in range(B_LOC)]                     # 2x 8 KB/part
        wv_sb = [singles.tile([P, CT, 512], DT, name=f"wv_{n}")
                 for n in range(2)]                        # 16 KB/part
        wo_sb = [singles.tile([P, CT, 512], DT, name=f"wo_{n}")
                 for n in range(2)]                        # 16 KB/part

        # per-(b,half) 1/l gather targets: row 32*kap+8*pp+s = head 2*pp+kap,
        # qt segment s (kap-major so a plain reshape DMA yields head rows)
        lgq = {(b, hf): singles.tile([DH, DH], DT, name=f"lgq{b}_{hf}")
               for b in range(B_LOC) for hf in range(2)}
        # [2,128] selector: broadcast head-row kap across partitions 64*kap..
        sel2 = singles.tile([2, P], DT)
        nc.gpsimd.dma_start(out=sel2[:], in_=sel2d)
        bias_sb = singles.tile([P, C], F32)                # 4 KB/part
        maskd_sb = singles.tile([P, NR, P], DT)            # 1 KB/part
        kpmb_sb = singles.tile([P, B_LOC * NR], F32)
        xk = [singles.tile([P, TOK], DT, name=f"x_{k}") for k in range(CT)]

        # --- prologue DMAs ---
        # First matmul needs wq0[:,0,:] + xk0[:,0:512]; split the big tiles
        # into halves and fan them across all three queues so the PE can
        # start ~4us earlier and is never gated on a single 256KB transfer.
        wq0 = wq_pool.tile([P, CT, P], DT, tag="wq", name="wq_0")
        nc.sync.dma_start(out=wq0[:, 0:4, :], in_=wqk[0, :, 0:4, :])
        nc.scalar.dma_start(out=xk[0][:, 0:512], in_=xT[0:P, 0:512])
        nc.gpsimd.dma_start(out=wq0[:, 4:8, :], in_=wqk[0, :, 4:8, :])
        # first halves of every xk (the tt=0 chain), spread over queues
        nc.sync.dma_start(out=xk[1][:, 0:512], in_=xT[P:2 * P, 0:512])
        nc.scalar.dma_start(out=xk[2][:, 0:512],
                            in_=xT[2 * P:3 * P, 0:512])
        nc.gpsimd.dma_start(out=xk[3][:, 0:512],
                            in_=xT[3 * P:4 * P, 0:512])
        nc.sync.dma_start(out=xk[4][:, 0:512], in_=xT[4 * P:5 * P, 0:512])
        nc.scalar.dma_start(out=xk[5][:, 0:512],
                            in_=xT[5 * P:6 * P, 0:512])
        nc.gpsimd.dma_start(out=xk[6][:, 0:512],
                            in_=xT[6 * P:7 * P, 0:512])
        nc.sync.dma_start(out=xk[7][:, 0:512], in_=xT[7 * P:8 * P, 0:512])
        # second halves (the tt=1 chain)
        for k in range(CT):
            eng = (nc.scalar, nc.gpsimd, nc.sync)[k % 3]
            eng.dma_start(out=xk[k][:, 512:1024],
                          in_=xT[k * P:(k + 1) * P, 512:1024])
        nc.scalar.dma_start(out=maskd_sb[:],
                            in_=maskd.rearrange("r p q -> p r q"))
        nc.scalar.dma_start(out=kpmb_sb[:],
                            in_=kpmb.rearrange("b (r p) -> p (b r)", p=P))
        nc.vector.memset(v_sb[:, :, :, DH:DH + 1], 1.0)

        def emit_qk(j, wq_tile=None):
            if wq_tile is None:
                wq_tile = wq_pool.tile([P, CT, P], DT, tag="wq",
                                       name=f"wq_{j}")
                nc.sync.dma_start(out=wq_tile[:], in_=wqk[j])
            for tt in range(2):
                ps = ps_qk.tile([P, 512], F32, tag="ps", name=f"ps_qk{j}_{tt}")
                for k in range(CT):
                    nc.tensor.matmul(ps[:], wq_tile[:, k, :],
                                     xk[k][:, tt * 512:(tt + 1) * 512],
                                     start=(k == 0), stop=(k == CT - 1))
                nc.vector.tensor_copy(
                    out=qk_sb[:, j, tt * 512:(tt + 1) * 512], in_=ps[:])

        pt_tiles = {}

        def emit_scores(b, h):
            p = h // 2
            dlo = DH * (h % 2)
            jq, jk = p, 8 + p
            pT = pt_pool.tile([P, PTW], DT, tag="pT", name=f"pT_{b}_{h}")
            pt_tiles[(b, h)] = pT
            sA = ps_s.tile([P, 512], F32, tag="sA", name=f"sA_{b}_{h}")
            sB = ps_s.tile([P, 512], F32, tag="sB", name=f"sB_{b}_{h}")
            sC = ps_s.tile([P, 512], F32, tag="sC", name=f"sC_{b}_{h}")
            sloc = [sA[:, 0:512], sB[:, 0:384], sC[:, 0:256], sC[:, 256:384]]
            for r in range(NR):
                ln = (NR - r) * P
                kT = qk_sb[dlo:dlo + DH, jk,
                           b * T + r * P: b * T + (r + 1) * P]
                qTr = qk_sb[dlo:dlo + DH, jq, b * T + r * P:(b + 1) * T]
                # r==3 continues r==2's group in sC: start=False does not
                # re-zero the bank, its bytes are still pending-zero from
                # r2's start, and the back-to-back same-bank matmuls
                # pipeline without a group-boundary bubble.
                nc.tensor.matmul(sloc[r], kT, qTr, start=(r != 3),
                                 stop=(r != 2), skip_group_check=(r == 3))
                if r == 2:
                    continue
                # one exp covers r2+r3 (their pT blocks are adjacent); kpm
                # is all-False in this problem (asserted host-side) so the
                # per-partition bias column is shared safely.
                lo = POFF[2] if r == 3 else POFF[r]
                src = sC[:, 0:384] if r == 3 else sloc[r]
                nc.scalar.activation(
                    out=pT[:, lo:POFF[r] + ln], in_=src,
                    func=mybir.ActivationFunctionType.Exp,
                    bias=kpmb_sb[:, b * NR + r: b * NR + r + 1])
                for rm in ([2, 3] if r == 3 else [r]):
                    nc.vector.tensor_mul(
                        out=pT[:, POFF[rm]:POFF[rm] + P],
                        in0=pT[:, POFF[rm]:POFF[rm] + P],
                        in1=maskd_sb[:, rm, :])

        def emit_v(n):
            for m in range(TOK // P):
                ps = ps_qk.tile([P, 512], F32, tag="ps", name=f"ps_v{n}_{m}")
                for k in range(CT):
                    nc.tensor.matmul(
                        ps[:], xk[k][:, m * P:(m + 1) * P], wv_sb[n][:, k, :],
                        start=(k == 0), stop=(k == CT - 1))
                nc.vector.tensor_copy(
                    out=v_sb[:, m, 8 * n:8 * n + 8, 0:DH],
                    in_=ps[:].rearrange("p (h d) -> p h d", d=DH))

        def emit_av_pair(b, p):
            half = p // 4
            for kap in range(2):
                h = 2 * p + kap
                pT = pt_tiles.pop((b, h))
                po = ps_o.tile([P, 512], F32, tag="po", name=f"po_{b}_{h}")
                for r in range(NR):
                    ln = (NR - r) * P
                    nc.tensor.matmul(po[0:DH + 1, r * P:],
                                     v_sb[:, b * NR + r, h, :],
                                     pT[:, POFF[r]:POFF[r] + ln],
                                     start=(r == 0), stop=(r == NR - 1))
                # drain all 65 rows (64 ao + the ones-column l row) in one
                # DVE op, then scatter by DMA: ao rows into ao_b, the l row
                # straight into the [64,64] reciprocal layout.
                st = aost_pool.tile([DH + 1, 512], DT, tag="aost",
                                    name=f"aost_{b}_{h}")
                nc.vector.tensor_copy(out=st[:], in_=po[0:DH + 1, :])
                rr = 32 * kap + 8 * (p % 4)
                lg = lgq[(b, half)]
                nc.sync.dma_start(out=lg[rr:rr + 8, :],
                                  in_=st[DH:DH + 1, :])
                eng = nc.gpsimd if (b + kap) % 2 else nc.sync
                eng.dma_start(
                    out=ao_b[b][kap * DH:(kap + 1) * DH, p, :],
                    in_=st[0:DH, :])

        def emit_norm_half(b, half):
            """1/l for 8 heads: reciprocal on the [64,64] gather (64 DVE
            lanes), cast, reshape-DMA into [2, 4, 512] head rows, then a
            K=2 selector matmul per pair broadcasts 1/l across the 128
            partitions in PSUM; the normalize mul reads it straight from
            there. No DRAM bounce, ~10KB of DMA instead of ~1MB."""
            qa = nc.sync if b == 0 else nc.gpsimd
            lg = lgq[(b, half)]
            liF = li_pool.tile([DH, DH], F32, tag="liF",
                               name=f"liF_{b}_{half}")
            nc.vector.reciprocal(out=liF[:], in_=lg[:])
            lpd = li_pool.tile([DH, DH], DT, tag="lpd",
                               name=f"lpd_{b}_{half}")
            nc.scalar.copy(out=lpd[:], in_=liF[:])
            lrow = li_pool.tile([2, 4, 512], DT, tag="lrow",
                                name=f"lrow_{b}_{half}")
            qa.dma_start(out=lrow[:], in_=lpd[:])
            for pp in range(4):
                lf = ps_o.tile([P, 512], F32, tag="po",
                               name=f"lf_{b}_{half}_{pp}")
                nc.tensor.matmul(lf[:], sel2[:], lrow[:, pp, :],
                                 start=True, stop=True)
                nc.vector.tensor_mul(
                    out=ao_b[b][:, half * 4 + pp, :],
                    in0=ao_b[b][:, half * 4 + pp, :],
                    in1=lf[:])

        out_qs = [nc.scalar, nc.sync, nc.gpsimd]

        def yproj_chunk(b, i, qi=[0]):
            n, m = i // NR, i % NR
            ps = ps_qk.tile([P, 512], F32, tag="ps", name=f"ps_y{b}_{n}_{m}")
            for k in range(CT):
                nc.tensor.matmul(ps[:], ao_b[b][:, k, m * P:(m + 1) * P],
                                 wo_sb[n][:, k, :],
                                 start=(k == 0), stop=(k == CT - 1))
            y = y_pool.tile([P, 512], DT, tag="y")
            nc.vector.tensor_add(out=y[:], in0=ps[:],
                                 in1=bias_sb[:, n * 512:(n + 1) * 512])
            out_qs[qi[0] % 3].dma_start(
                out=out[b * T + m * P: b * T + (m + 1) * P,
                        n * 512:(n + 1) * 512],
                in_=y[:])
            qi[0] += 1

        psv_tiles = {}

        def yproj_stage1(b, i):
            """k0..3 partial (heads 0..7: normalized early) + bias, parked
            in SBUF; bridges the PE over the half-1 norm chains."""
            n, m = i // NR, i % NR
            ps = ps_qk.tile([P, 512], F32, tag="ps", name=f"ps_y1{b}_{n}_{m}")
            for k in range(4):
                nc.tensor.matmul(ps[:], ao_b[b][:, k, m * P:(m + 1) * P],
                                 wo_sb[n][:, k, :],
                                 start=(k == 0), stop=(k == 3))
            sv = psv_pool.tile([P, 512], DT, tag="psv", name=f"psv{b}_{i}")
            psv_tiles[(b, i)] = sv
            nc.vector.tensor_add(out=sv[:], in0=ps[:],
                                 in1=bias_sb[:, n * 512:(n + 1) * 512])

        def yproj_stage2(b, i, qi=[0]):
            n, m = i // NR, i % NR
            ps = ps_qk.tile([P, 512], F32, tag="ps", name=f"ps_y2{b}_{n}_{m}")
            for k in range(4, CT):
                nc.tensor.matmul(ps[:], ao_b[b][:, k, m * P:(m + 1) * P],
                                 wo_sb[n][:, k, :],
                                 start=(k == 4), stop=(k == CT - 1))
            y = y_pool.tile([P, 512], DT, tag="y")
            nc.vector.tensor_add(out=y[:], in0=ps[:],
                                 in1=psv_tiles.pop((b, i))[:])
            out_qs[qi[0] % 3].dma_start(
                out=out[b * T + m * P: b * T + (m + 1) * P,
                        n * 512:(n + 1) * 512],
                in_=y[:])
            qi[0] += 1

        # --- main interleaved schedule ---
        for p in range(8):
            emit_qk(p, wq0 if p == 0 else None)
            emit_qk(8 + p)
            for b in range(B_LOC):
                for kap in range(2):
                    emit_scores(b, 2 * p + kap)
            if p == 0:
                nc.scalar.dma_start(out=wv_sb[0][:], in_=wv[:, 0])
            elif p == 1:
                emit_v(0)
                nc.scalar.dma_start(out=wv_sb[1][:], in_=wv[:, 1])
                for b in range(B_LOC):
                    emit_av_pair(b, 0)
            elif p == 2:
                for b in range(B_LOC):
                    emit_av_pair(b, 1)
            elif p == 3:
                emit_v(1)
                for b in range(B_LOC):
                    emit_av_pair(b, 2)
            elif p == 4:
                for b in range(B_LOC):
                    emit_av_pair(b, 3)
                for n in range(2):
                    nc.scalar.dma_start(out=wo_sb[n][:], in_=wo[:, n])
                bias_bcast = bass.AP(tensor=bias.tensor, offset=bias.offset,
                                     ap=[[0, P], *bias.ap])
                nc.gpsimd.dma_start(out=bias_sb[:], in_=bias_bcast)
            elif p == 5:
                for b in range(B_LOC):
                    emit_av_pair(b, 4)
            elif p == 6:
                for b in range(B_LOC):
                    emit_norm_half(b, 0)
                for b in range(B_LOC):
                    emit_av_pair(b, 5)
            elif p == 7:
                for b in range(B_LOC):
                    emit_av_pair(b, 6)
        # tail: split-k b=0 chunks (k0..3 = heads 0..7, normalized early)
        # keep the PE busy while the half-1 norm chains resolve
        emit_av_pair(0, 7)
        yproj_stage1(0, 0)
        yproj_stage1(0, 1)
        emit_norm_half(0, 1)
        emit_av_pair(1, 7)
        yproj_stage1(0, 2)
        yproj_stage1(0, 3)
        emit_norm_half(1, 1)
        yproj_stage1(0, 4)
        yproj_stage1(0, 5)
        for i in range(N_ST1):
            yproj_stage2(0, i)
        for i in range(N_ST1, 2 * NR):
            yproj_chunk(0, i)
        for i in range(2 * NR):
            yproj_chunk(1, i)


_NC_CACHE = None


def _get_nc():
    global _NC_CACHE
    if _NC_CACHE is None:
        _NC_CACHE = _build_nc()
    return _NC_CACHE


def _prep_core_inputs(x, mask, key_padding_mask, w_qkv, w_out, b_out):
    """Host-side sharding + layout prep. Returns list of per-core in_maps."""
    x = np.asarray(x, dtype=np.float32)
    mask = np.asarray(mask)
    kpm = np.asarray(key_padding_mask)
    w_qkv = np.asarray(w_qkv, dtype=np.float32)
    w_out = np.asarray(w_out, dtype=np.float32)
    b_out = np.asarray(b_out, dtype=np.float32)

    scale = 1.0 / math.sqrt(DH)
    w2 = w_qkv[:2 * C].copy()
    w2[:C] *= scale  # fold 1/sqrt(dh) into the Q weights
    # [j, p, k, f]: contiguous 2KB/partition DMA per j-tile
    wqk_r = np.ascontiguousarray(
        w2.reshape(16, P, CT, P).transpose(0, 3, 2, 1).astype(np.float16))
    # wv/wo as [p, n, k, f]: contiguous per-partition lines
    wv_r = np.ascontiguousarray(
        w_qkv[2 * C:].T.reshape(CT, P, 2, 512).transpose(1, 2, 0, 3)
        .astype(np.float16))
    wo_r = np.ascontiguousarray(
        w_out.T.reshape(CT, P, 2, 512).transpose(1, 2, 0, 3)
        .astype(np.float16))

    # The kernel exploits the causal structure: it only applies mask values
    # inside the diagonal 128x128 blocks and zero-fills fully-masked blocks.
    exp_tril = np.tril(np.ones((T, T), dtype=mask.dtype))
    assert np.array_equal(mask, exp_tril), "kernel assumes causal tril mask"
    # the merged r2+r3 exp shares one kpm bias column; only valid when no
    # key positions are padded (always true for this problem's inputs)
    assert not kpm.any(), "kernel assumes all-False key_padding_mask"
    maskTf = mask.T.astype(np.float16)  # [kt, qt]
    maskd = np.stack([maskTf[r * P:(r + 1) * P, r * P:(r + 1) * P]
                      for r in range(NR)])  # [NR, P, P]

    # [2,128] selector for the 1/l partition broadcast matmul
    sel2_h = np.zeros((2, P), dtype=np.float16)
    sel2_h[0, 0:DH] = 1.0
    sel2_h[1, DH:P] = 1.0

    in_maps = []
    for i in range(N_CORES):
        xs = x[i * B_LOC:(i + 1) * B_LOC]      # [B_LOC, T, C]
        xT = np.ascontiguousarray(xs.reshape(TOK, C).T.astype(np.float16))
        kb = np.where(kpm[i * B_LOC:(i + 1) * B_LOC], -1e30,
                      0.0).astype(np.float32)  # [B_LOC, T]
        in_maps.append({
            "xT": xT,
            "wqk": wqk_r,
            "wv": wv_r,
            "wo": wo_r,
            "maskd": np.ascontiguousarray(maskd),
            "kpmb": kb,
            "bias": b_out,
            "sel2d": sel2_h,
        })
    return in_maps


def kernel(x, mask, key_padding_mask, w_qkv, w_out, b_out, _trace=False,
           _tmpdir=None):
    nc = _get_nc()
    in_maps = _prep_core_inputs(x, mask, key_padding_mask, w_qkv, w_out, b_out)
    res = run_bass_kernel_spmd(nc, in_maps, list(range(N_CORES)),
                               trace=_trace, tmpdir=_tmpdir)
    outs = [np.asarray(res.results[i]["out"], dtype=np.float32)
            .reshape(B_LOC, T, C) for i in range(N_CORES)]
    full = np.concatenate(outs, axis=0)
    kernel._last_exec_time_ns = res.exec_time_ns
    return full


# revision 28
# speedup vs baseline: 1.0387x; 1.0028x over previous
"""Multi-head self-attention (B=16,T=512,C=1024,H=16) on 8 NeuronCores.

Strategy: data-parallel over batch (2 batches/core), no collectives.
Schedule keeps the PE dense (HAM stays warm at 2.4GHz) and hides the
scalar-engine exp (the softmax) under the projection matmuls:

  pair p: QK-project head pair p -> scores + exp + mask for 4 (b,h) chains
  V projection + AV + out-projection woven between pairs so every engine
  always has work and nothing big sits at the tail.

Layouts avoid on-device transposes (same tricks as the ancestor kernel):
  - QK projection emits [f, tok]; scores are computed transposed
    sT[kt, qt]; softmax sums come from a ones-column appended to v.
  - AV drains move all 65 PSUM rows (64 ao + 1 l-row) to a staging tile;
    SBUF->SBUF DMAs then scatter the ao rows into place and the l-row
    straight into the [64,64] reciprocal layout (lgq), keeping the l
    bookkeeping off the ACT engine entirely.
  - softmax 1/l: reciprocal on the [64,64] tile (64 DVE lanes), cast,
    DRAM write + stride-0 broadcast read-back, one fused [128,4,512]
    normalize mul per (b,half); b=0 chains ride the sync DMA queue and
    b=1 the gpsimd queue so the two tails overlap.
  - out-projection: the first 6 chunks of b=0 run split-k: k0..3 (heads
    0..7, normalized early) accumulate while the half-1 norm chains
    drain, then k4..7 finish. The PE never idles over the norm latency.
Engine roles: PE matmuls only; scalar softmax exp; vector drains, diag
masks, reciprocal, normalize; sync + scalar + gpsimd DMA queues split
so x/weights, the l-chain and out-writes never serialize behind each
other. PSUM: 2 banks QK/V/yproj + 3 scores (r1/r3 share a bank via a
start=False accumulate into the still-pending zero-region) + 3 AV.
"""

import math
from contextlib import ExitStack

import numpy as np

import concourse.bass as bass
import concourse.mybir as mybir
import concourse.tile as tile
from concourse import bacc
from concourse.bass_utils import run_bass_kernel_spmd

N_CORES = 8
B, T, C = 16, 512, 1024
H = 16
DH = C // H  # 64
B_LOC = B // N_CORES  # 2
TOK = B_LOC * T  # 1024 tokens per core
P = 128
CT = C // P  # 8 contraction tiles
NR = T // P  # 4 kt blocks
DT = mybir.dt.float16
F32 = mybir.dt.float32

# compact pT column offsets per kt-block r (lengths 512,384,256,128)
POFF = [0, 512, 896, 1152]
PTW = 1280

N_ST1 = 6  # b=0 out-proj chunks run as split-k (bridge the norm latency)


def _build_nc():
    nc = bacc.Bacc("TRN2", target_bir_lowering=False, debug=False,
                   num_devices=N_CORES)

    xT = nc.dram_tensor("xT", [C, TOK], DT, kind="ExternalInput").ap()
    wqk = nc.dram_tensor("wqk", [16, P, CT, P], DT, kind="ExternalInput").ap()
    wv = nc.dram_tensor("wv", [P, 2, CT, 512], DT, kind="ExternalInput").ap()
    wo = nc.dram_tensor("wo", [P, 2, CT, 512], DT, kind="ExternalInput").ap()
    maskd = nc.dram_tensor("maskd", [NR, P, P], DT,
                           kind="ExternalInput").ap()
    kpmb = nc.dram_tensor("kpmb", [B_LOC, T], F32, kind="ExternalInput").ap()
    bias = nc.dram_tensor("bias", [C], F32, kind="ExternalInput").ap()
    sel2d = nc.dram_tensor("sel2d", [2, P], DT, kind="ExternalInput").ap()
    out = nc.dram_tensor("out", [TOK, C], DT, kind="ExternalOutput").ap()

    with tile.TileContext(nc) as tc:
        _emit(nc, tc, xT, wqk, wv, wo, maskd, kpmb, bias, sel2d, out)

    nc.compile()
    return nc


def _emit(nc, tc, xT, wqk, wv, wo, maskd, kpmb, bias, sel2d, out):
    ctx = ExitStack()
    with ctx:
        singles = ctx.enter_context(tc.tile_pool(name="singles", bufs=1))
        ps_qk = ctx.enter_context(tc.tile_pool(name="ps_qk", bufs=2,
                                               space="PSUM"))
        ps_s = ctx.enter_context(tc.tile_pool(name="ps_s", bufs=1,
                                              space="PSUM"))
        ps_o = ctx.enter_context(tc.tile_pool(name="ps_o", bufs=3,
                                              space="PSUM"))
        wq_pool = ctx.enter_context(tc.tile_pool(name="wq", bufs=4))
        pt_pool = ctx.enter_context(tc.tile_pool(name="pt", bufs=11))
        li_pool = ctx.enter_context(tc.tile_pool(name="li", bufs=2))
        aost_pool = ctx.enter_context(tc.tile_pool(name="aost", bufs=4))
        psv_pool = ctx.enter_context(tc.tile_pool(name="psv",
                                                  bufs=2 * N_ST1))
        y_pool = ctx.enter_context(tc.tile_pool(name="y", bufs=3))

        # --- persistent SBUF tensors ---
        qk_sb = singles.tile([P, 16, TOK], DT)             # 32 KB/part
        v_sb = singles.tile([P, TOK // P, H, DH + 1], DT)  # 16.6 KB/part
        ao_b = [singles.tile([P, CT, T], DT, name=f"ao_b{b}")
                for b in range(B_LOC)]                     # 2x 8 KB/part
        wv_sb = [singles.tile([P, CT, 512], DT, name=f"wv_{n}")
                 for n in range(2)]                        # 16 KB/part
        wo_sb = [singles.tile([P, CT, 512], DT, name=f"wo_{n}")
                 for n in range(2)]                        # 16 KB/part

        # per-(b,half) 1/l gather targets: row 32*kap+8*pp+s = head 2*pp+kap,
        # qt segment s (kap-major so a plain reshape DMA yields head rows)
        lgq = {(b, hf): singles.tile([DH, DH], DT, name=f"lgq{b}_{hf}")
               for b in range(B_LOC) for hf in range(2)}
        # [2,128] selector: broadcast head-row kap across partitions 64*kap..
        sel2 = singles.tile([2, P], DT)
        nc.gpsimd.dma_start(out=sel2[:], in_=sel2d)
        bias_sb = singles.tile([P, C], F32)                # 4 KB/part
        maskd_sb = singles.tile([P, NR, P], DT)            # 1 KB/part
        kpmb_sb = singles.tile([P, B_LOC * NR], F32)
        xk = [singles.tile([P, TOK], DT, name=f"x_{k}") for k in range(CT)]

        # --- prologue DMAs ---
        # First matmul needs wq0[:,0,:] + xk0[:,0:512]; split the big tiles
        # into halves and fan them across all three queues so the PE can
        # start ~4us earlier and is never gated on a single 256KB transfer.
        wq0 = wq_pool.tile([P, CT, P], DT, tag="wq", name="wq_0")
        nc.sync.dma_start(out=wq0[:, 0:4, :], in_=wqk[0, :, 0:4, :])
        nc.scalar.dma_start(out=xk[0][:, 0:512], in_=xT[0:P, 0:512])
        nc.gpsimd.dma_start(out=wq0[:, 4:8, :], in_=wqk[0, :, 4:8, :])
        # first halves of every xk (the tt=0 chain), spread over queues
        nc.sync.dma_start(out=xk[1][:, 0:512], in_=xT[P:2 * P, 0:512])
        nc.scalar.dma_start(out=xk[2][:, 0:512],
                            in_=xT[2 * P:3 * P, 0:512])
        nc.gpsimd.dma_start(out=xk[3][:, 0:512],
                            in_=xT[3 * P:4 * P, 0:512])
        nc.sync.dma_start(out=xk[4][:, 0:512], in_=xT[4 * P:5 * P, 0:512])
        nc.scalar.dma_start(out=xk[5][:, 0:512],
                            in_=xT[5 * P:6 * P, 0:512])
        nc.gpsimd.dma_start(out=xk[6][:, 0:512],
                            in_=xT[6 * P:7 * P, 0:512])
        nc.sync.dma_start(out=xk[7][:, 0:512], in_=xT[7 * P:8 * P, 0:512])
        # second halves (the tt=1 chain)
        for k in range(CT):
            eng = (nc.scalar, nc.gpsimd, nc.sync)[k % 3]
            eng.dma_start(out=xk[k][:, 512:1024],
                          in_=xT[k * P:(k + 1) * P, 512:1024])
        nc.scalar.dma_start(out=maskd_sb[:],
                            in_=maskd.rearrange("r p q -> p r q"))
        nc.scalar.dma_start(out=kpmb_sb[:],
                            in_=kpmb.rearrange("b (r p) -> p (b r)", p=P))
        nc.vector.memset(v_sb[:, :, :, DH:DH + 1], 1.0)

        def emit_qk(j, wq_tile=None):
            if wq_tile is None:
                wq_tile = wq_pool.tile([P, CT, P], DT, tag="wq",
                                       name=f"wq_{j}")
                nc.sync.dma_start(out=wq_tile[:], in_=wqk[j])
            for tt in range(2):
                ps = ps_qk.tile([P, 512], F32, tag="ps", name=f"ps_qk{j}_{tt}")
                for k in range(CT):
                    nc.tensor.matmul(ps[:], wq_tile[:, k, :],
                                     xk[k][:, tt * 512:(tt + 1) * 512],
                                     start=(k == 0), stop=(k == CT - 1))
                nc.vector.tensor_copy(
                    out=qk_sb[:, j, tt * 512:(tt + 1) * 512], in_=ps[:])

        pt_tiles = {}

        def emit_scores(b, h):
            p = h // 2
            dlo = DH * (h % 2)
            jq, jk = p, 8 + p
            pT = pt_pool.tile([P, PTW], DT, tag="pT", name=f"pT_{b}_{h}")
            pt_tiles[(b, h)] = pT
            sA = ps_s.tile([P, 512], F32, tag="sA", name=f"sA_{b}_{h}")
            sB = ps_s.tile([P, 512], F32, tag="sB", name=f"sB_{b}_{h}")
            sC = ps_s.tile([P, 512], F32, tag="sC", name=f"sC_{b}_{h}")
            sloc = [sA[:, 0:512], sB[:, 0:384], sC[:, 0:256], sC[:, 256:384]]
            for r in range(NR):
                ln = (NR - r) * P
                kT = qk_sb[dlo:dlo + DH, jk,
                           b * T + r * P: b * T + (r + 1) * P]
                qTr = qk_sb[dlo:dlo + DH, jq, b * T + r * P:(b + 1) * T]
                # r==3 continues r==2's group in sC: start=False does not
                # re-zero the bank, its bytes are still pending-zero from
                # r2's start, and the back-to-back same-bank matmuls
                # pipeline without a group-boundary bubble.
                nc.tensor.matmul(sloc[r], kT, qTr, start=(r != 3),
                                 stop=(r != 2), skip_group_check=(r == 3))
                if r == 2:
                    continue
                # one exp covers r2+r3 (their pT blocks are adjacent); kpm
                # is all-False in this problem (asserted host-side) so the
                # per-partition bias column is shared safely.
                lo = POFF[2] if r == 3 else POFF[r]
                src = sC[:, 0:384] if r == 3 else sloc[r]
                nc.scalar.activation(
                    out=pT[:, lo:POFF[r] + ln], in_=src,
                    func=mybir.ActivationFunctionType.Exp,
                    bias=kpmb_sb[:, b * NR + r: b * NR + r + 1])
                for rm in ([2, 3] if r == 3 else [r]):
                    nc.vector.tensor_mul(
                        out=pT[:, POFF[rm]:POFF[rm] + P],
                        in0=pT[:, POFF[rm]:POFF[rm] + P],
                        in1=maskd_sb[:, rm, :])

        def emit_v(n):
            for m in range(TOK // P):
                ps = ps_qk.tile([P, 512], F32, tag="ps", name=f"ps_v{n}_{m}")
                for k in range(CT):
                    nc.tensor.matmul(
                        ps[:], xk[k][:, m * P:(m + 1) * P], wv_sb[n][:, k, :],
                        start=(k == 0), stop=(k == CT - 1))
                nc.vector.tensor_copy(
                    out=v_sb[:, m, 8 * n:8 * n + 8, 0:DH],
                    in_=ps[:].rearrange("p (h d) -> p h d", d=DH))

        def emit_av_pair(b, p):
            half = p // 4
            for kap in range(2):
                h = 2 * p + kap
                pT = pt_tiles.pop((b, h))
                po = ps_o.tile([P, 512], F32, tag="po", name=f"po_{b}_{h}")
                for r in range(NR):
                    ln = (NR - r) * P
                    nc.tensor.matmul(po[0:DH + 1, r * P:],
                                     v_sb[:, b * NR + r, h, :],
                                     pT[:, POFF[r]:POFF[r] + ln],
                                     start=(r == 0), stop=(r == NR - 1))
                # drain all 65 rows (64 ao + the ones-column l row) in one
                # DVE op, then scatter by DMA: ao rows into ao_b, the l row
                # straight into the [64,64] reciprocal layout.
                st = aost_pool.tile([DH + 1, 512], DT, tag="aost",
                                    name=f"aost_{b}_{h}")
                nc.vector.tensor_copy(out=st[:], in_=po[0:DH + 1, :])
                rr = 32 * kap + 8 * (p % 4)
                lg = lgq[(b, half)]
                nc.sync.dma_start(out=lg[rr:rr + 8, :],
                                  in_=st[DH:DH + 1, :])
                eng = nc.gpsimd if (b + kap) % 2 else nc.sync
                eng.dma_start(
                    out=ao_b[b][kap * DH:(kap + 1) * DH, p, :],
                    in_=st[0:DH, :])

        def emit_norm_half(b, half):
            """1/l for 8 heads: reciprocal on the [64,64] gather (64 DVE
            lanes), cast, reshape-DMA into [2, 4, 512] head rows, then a
            K=2 selector matmul per pair broadcasts 1/l across the 128
            partitions in PSUM; the normalize mul reads it straight from
            there. No DRAM bounce, ~10KB of DMA instead of ~1MB."""
            qa = nc.sync if b == 0 else nc.gpsimd
            lg = lgq[(b, half)]
            liF = li_pool.tile([DH, DH], F32, tag="liF",
                               name=f"liF_{b}_{half}")
            nc.vector.reciprocal(out=liF[:], in_=lg[:])
            lpd = li_pool.tile([DH, DH], DT, tag="lpd",
                               name=f"lpd_{b}_{half}")
            nc.scalar.copy(out=lpd[:], in_=liF[:])
            lrow = li_pool.tile([2, 4, 512], DT, tag="lrow",
                                name=f"lrow_{b}_{half}")
            qa.dma_start(out=lrow[:], in_=lpd[:])
            for pp in range(4):
                lf = ps_o.tile([P, 512], F32, tag="po",
                               name=f"lf_{b}_{half}_{pp}")
                nc.tensor.matmul(lf[:], sel2[:], lrow[:, pp, :],
                                 start=True, stop=True)
                nc.vector.tensor_mul(
                    out=ao_b[b][:, half * 4 + pp, :],
                    in0=ao_b[b][:, half * 4 + pp, :],
                    in1=lf[:])

        out_qs = [nc.scalar, nc.sync, nc.gpsimd]

        def yproj_chunk(b, i, qi=[0]):
            n, m = i // NR, i % NR
            ps = ps_qk.tile([P, 512], F32, tag="ps", name=f"ps_y{b}_{n}_{m}")
            for k in range(CT):
                nc.tensor.matmul(ps[:], ao_b[b][:, k, m * P:(m + 1) * P],
                                 wo_sb[n][:, k, :],
                                 start=(k == 0), stop=(k == CT - 1))
            y = y_pool.tile([P, 512], DT, tag="y")
            nc.vector.tensor_add(out=y[:], in0=ps[:],
                                 in1=bias_sb[:, n * 512:(n + 1) * 512])
            out_qs[qi[0] % 3].dma_start(
                out=out[b * T + m * P: b * T + (m + 1) * P,
                        n * 512:(n + 1) * 512],
                in_=y[:])
            qi[0] += 1

        psv_tiles = {}

        def yproj_stage1(b, i):
            """k0..3 partial (heads 0..7: normalized early) + bias, parked
            in SBUF; bridges the PE over the half-1 norm chains."""
            n, m = i // NR, i % NR
            ps = ps_qk.tile([P, 512], F32, tag="ps", name=f"ps_y1{b}_{n}_{m}")
            for k in range(4):
                nc.tensor.matmul(ps[:], ao_b[b][:, k, m * P:(m + 1) * P],
                                 wo_sb[n][:, k, :],
                                 start=(k == 0), stop=(k == 3))
            sv = psv_pool.tile([P, 512], DT, tag="psv", name=f"psv{b}_{i}")
            psv_tiles[(b, i)] = sv
            nc.vector.tensor_add(out=sv[:], in0=ps[:],
                                 in1=bias_sb[:, n * 512:(n + 1) * 512])

        def yproj_stage2(b, i, qi=[0]):
            n, m = i // NR, i % NR
            ps = ps_qk.tile([P, 512], F32, tag="ps", name=f"ps_y2{b}_{n}_{m}")
            for k in range(4, CT):
                nc.tensor.matmul(ps[:], ao_b[b][:, k, m * P:(m + 1) * P],
                                 wo_sb[n][:, k, :],
                                 start=(k == 4), stop=(k == CT - 1))
            y = y_pool.tile([P, 512], DT, tag="y")
            nc.vector.tensor_add(out=y[:], in0=ps[:],
                                 in1=psv_tiles.pop((b, i))[:])
            out_qs[qi[0] % 3].dma_start(
                out=out[b * T + m * P: b * T + (m + 1) * P,
                        n * 512:(n + 1) * 512],
                in_=y[:])
            qi[0] += 1

        # --- main interleaved schedule ---
        for p in range(8):
            emit_qk(p, wq0 if p == 0 else None)
            emit_qk(8 + p)
            for b in range(B_LOC):
                for kap in range(2):
                    emit_scores(b, 2 * p + kap)
            if p == 0:
                nc.scalar.dma_start(out=wv_sb[0][:], in_=wv[:, 0])
            elif p == 1:
                emit_v(0)
                nc.scalar.dma_start(out=wv_sb[1][:], in_=wv[:, 1])
                for b in range(B_LOC):
                    emit_av_pair(b, 0)
            elif p == 2:
                for b in range(B_LOC):
                    emit_av_pair(b, 1)
            elif p == 3:
                emit_v(1)
                for b in range(B_LOC):
                    emit_av_pair(b, 2)
            elif p == 4:
                for b in range(B_LOC):
                    emit_av_pair(b, 3)
                for n in range(2):
                    nc.scalar.dma_start(out=wo_sb[n][:], in_=wo[:, n])
                bias_bcast = bass.AP(tensor=bias.tensor, offset=bias.offset,
                                     ap=[[0, P], *bias.ap])
                nc.gpsimd.dma_start(out=bias_sb[:], in_=bias_bcast)
            elif p == 5:
                for b in range(B_LOC):
                    emit_av_pair(b, 4)
            elif p == 6:
                for b in range(B_LOC):
                    emit_norm_half(b, 0)
                for b in range(B_LOC):
                    emit_av_pair(b, 5)
            elif p == 7:
                # b=1 split-k chunks (k0..3: heads 0..7, normalized at p=6)
                # pad the boundary-bubble-heavy last stage with dense 4-mm
                # chains and shrink the serial tail
                yproj_stage1(1, 0)
                yproj_stage1(1, 1)
                for b in range(B_LOC):
                    emit_av_pair(b, 6)
                for i in range(2, N_ST1):
                    yproj_stage1(1, i)
        # tail: split-k b=0 chunks (k0..3 = heads 0..7, normalized early)
        # keep the PE busy while the half-1 norm chains resolve
        emit_av_pair(0, 7)
        yproj_stage1(0, 0)
        yproj_stage1(0, 1)
        emit_norm_half(0, 1)
        emit_av_pair(1, 7)
        yproj_stage1(0, 2)
        yproj_stage1(0, 3)
        emit_norm_half(1, 1)
        yproj_stage1(0, 4)
        yproj_stage1(0, 5)
        for i in range(N_ST1):
            yproj_stage2(0, i)
        for i in range(N_ST1, 2 * NR):
            yproj_chunk(0, i)
        for i in range(N_ST1):
            yproj_stage2(1, i)
        for i in range(N_ST1, 2 * NR):
            yproj_chunk(1, i)


_NC_CACHE = None


def _get_nc():
    global _NC_CACHE
    if _NC_CACHE is None:
        _NC_CACHE = _build_nc()
    return _NC_CACHE


def _prep_core_inputs(x, mask, key_padding_mask, w_qkv, w_out, b_out):
    """Host-side sharding + layout prep. Returns list of per-core in_maps."""
    x = np.asarray(x, dtype=np.float32)
    mask = np.asarray(mask)
    kpm = np.asarray(key_padding_mask)
    w_qkv = np.asarray(w_qkv, dtype=np.float32)
    w_out = np.asarray(w_out, dtype=np.float32)
    b_out = np.asarray(b_out, dtype=np.float32)

    scale = 1.0 / math.sqrt(DH)
    w2 = w_qkv[:2 * C].copy()
    w2[:C] *= scale  # fold 1/sqrt(dh) into the Q weights
    # [j, p, k, f]: contiguous 2KB/partition DMA per j-tile
    wqk_r = np.ascontiguousarray(
        w2.reshape(16, P, CT, P).transpose(0, 3, 2, 1).astype(np.float16))
    # wv/wo as [p, n, k, f]: contiguous per-partition lines
    wv_r = np.ascontiguousarray(
        w_qkv[2 * C:].T.reshape(CT, P, 2, 512).transpose(1, 2, 0, 3)
        .astype(np.float16))
    wo_r = np.ascontiguousarray(
        w_out.T.reshape(CT, P, 2, 512).transpose(1, 2, 0, 3)
        .astype(np.float16))

    # The kernel exploits the causal structure: it only applies mask values
    # inside the diagonal 128x128 blocks and zero-fills fully-masked blocks.
    exp_tril = np.tril(np.ones((T, T), dtype=mask.dtype))
    assert np.array_equal(mask, exp_tril), "kernel assumes causal tril mask"
    # the merged r2+r3 exp shares one kpm bias column; only valid when no
    # key positions are padded (always true for this problem's inputs)
    assert not kpm.any(), "kernel assumes all-False key_padding_mask"
    maskTf = mask.T.astype(np.float16)  # [kt, qt]
    maskd = np.stack([maskTf[r * P:(r + 1) * P, r * P:(r + 1) * P]
                      for r in range(NR)])  # [NR, P, P]

    # [2,128] selector for the 1/l partition broadcast matmul
    sel2_h = np.zeros((2, P), dtype=np.float16)
    sel2_h[0, 0:DH] = 1.0
    sel2_h[1, DH:P] = 1.0

    in_maps = []
    for i in range(N_CORES):
        xs = x[i * B_LOC:(i + 1) * B_LOC]      # [B_LOC, T, C]
        xT = np.ascontiguousarray(xs.reshape(TOK, C).T.astype(np.float16))
        kb = np.where(kpm[i * B_LOC:(i + 1) * B_LOC], -1e30,
                      0.0).astype(np.float32)  # [B_LOC, T]
        in_maps.append({
            "xT": xT,
            "wqk": wqk_r,
            "wv": wv_r,
            "wo": wo_r,
            "maskd": np.ascontiguousarray(maskd),
            "kpmb": kb,
            "bias": b_out,
            "sel2d": sel2_h,
        })
    return in_maps


def kernel(x, mask, key_padding_mask, w_qkv, w_out, b_out, _trace=False,
           _tmpdir=None):
    nc = _get_nc()
    in_maps = _prep_core_inputs(x, mask, key_padding_mask, w_qkv, w_out, b_out)
    res = run_bass_kernel_spmd(nc, in_maps, list(range(N_CORES)),
                               trace=_trace, tmpdir=_tmpdir)
    outs = [np.asarray(res.results[i]["out"], dtype=np.float32)
            .reshape(B_LOC, T, C) for i in range(N_CORES)]
    full = np.concatenate(outs, axis=0)
    kernel._last_exec_time_ns = res.exec_time_ns
    return full
